# revision 1
# baseline (speedup 1.0000x reference)
"""Self-contained Trainium2 Bass kernel for nn_CrossLayerBlock (MoE routing).

8-way token parallelism; uniform SPMD program via per-core token permutation
(own tokens first). Causality = data-driven key-bias rows folded into the score
matmul + 4 universal diagonal mask tiles. Router in true fp32; big matmuls in
float32r; attention probabilities/V in bf16. MoE: global capacity via counts
AllGather + prefix-sum matmuls; kept rows scattered per-expert (indirect DMA),
dense per-expert MLP on compacted buffers, indirect gather back.
"""
import numpy as np
from contextlib import ExitStack

import concourse.bass as bass
import concourse.tile as tile
from concourse import bacc, mybir
from concourse import bass_utils
from concourse.masks import make_identity

B, T, D, H, HS, E = 4, 2048, 768, 12, 64, 8
NCORES = 8
NOWN = 1024
NKC = 16
DC = 6
SE = 192
ZROW = E * SE
LN_EPS = 1e-5
NEG = -30.0

f32 = mybir.dt.float32
f32r = mybir.dt.float32r
bf16 = mybir.dt.bfloat16
f8 = mybir.dt.float8e4
WSCALE = 64.0
XSCALE = 16.0
i32 = mybir.dt.int32
u32 = mybir.dt.uint32
AF = mybir.ActivationFunctionType
ALU = mybir.AluOpType

KCS0 = [8, 9, 10, 11, 0, 1, 2, 3]
KCS1 = [8, 9, 10, 11, 12, 13, 14, 15, 0, 1, 2, 3, 4, 5, 6, 7]


def _ln(nc, pool, xt, eps_col, scratch):
    # var = (E[x^2] - mu^2): x^2 path on gpsimd, in parallel with the
    # vector-side sum; normalize with a single fused (x-mu)*rstd op.
    r = pool.tile([128, 1], f32, tag="ln_r", name="ln_r")
    nc.vector.reduce_sum(r[:], xt[:], axis=mybir.AxisListType.X)
    sq = scratch
    r2 = pool.tile([128, 1], f32, tag="ln_r2", name="ln_r2")
    nc.gpsimd.tensor_tensor(out=sq[:], in0=xt[:], in1=xt[:], op=ALU.mult)
    nc.vector.reduce_sum(r2[:], sq[:], axis=mybir.AxisListType.X)
    mu = pool.tile([128, 1], f32, tag="ln_mu", name="ln_mu")
    nc.vector.tensor_scalar_mul(mu[:], r[:], 1.0 / D)
    mr = pool.tile([128, 1], f32, tag="ln_mr", name="ln_mr")
    nc.vector.tensor_tensor(out=mr[:], in0=mu[:], in1=r[:], op=ALU.mult)
    vd = pool.tile([128, 1], f32, tag="ln_vd", name="ln_vd")
    nc.vector.tensor_sub(vd[:], r2[:], mr[:])
    sd = pool.tile([128, 1], f32, tag="ln_sd", name="ln_sd")
    nc.scalar.activation(sd[:], vd[:], AF.Sqrt, bias=eps_col[:, :1],
                         scale=1.0 / D)
    rstd = pool.tile([128, 1], f32, tag="ln_rstd", name="ln_rstd")
    nc.vector.reciprocal(rstd[:], sd[:])
    xn = pool.tile([128, D], f32, tag="ln_xn", name="ln_xn")
    nc.vector.tensor_scalar(xn[:], xt[:], mu[:, :1], rstd[:, :1],
                            ALU.subtract, ALU.mult)
    return xn


def build_program():
    nc = bacc.Bacc("TRN2", target_bir_lowering=False, debug=False,
                   enable_asserts=False, num_devices=NCORES)

    din = {}
    for name, shape, dt in [
        ("xp", [T, D], f32), ("noise", [NOWN, E], f32),
        ("wq", [D, D], f32), ("wk", [D, D], f32), ("wv", [D, D], f32),
        ("wo", [D, D], f32), ("wrout", [D, 17], f32), ("rbias", [1, 17], f32),
        ("we1", [E, 6, 128, 3072], f8), ("we2", [E, 6, 128, 3072], f8),
        ("kbias", [2, T], f32), ("qsel", [2, NOWN], f32),
        ("chunksel", [64, E], f32),
    ]:
        din[name] = nc.dram_tensor(name, shape, dt, kind="ExternalInput").ap()

    yout = nc.dram_tensor("yout", [NOWN, D], f32, kind="ExternalOutput").ap()
    x1dbg = nc.dram_tensor("x1dbg", [NOWN, D], f32, kind="ExternalOutput").ap()
    rdbg = nc.dram_tensor("rdbg", [NOWN, E], f32, kind="ExternalOutput").ap()

    with tile.TileContext(nc) as tc, ExitStack() as top:
        dram = top.enter_context(tc.tile_pool(name="dram", bufs=1, space="DRAM"))
        xe_dram = dram.tile([E * SE, D], f32)
        ye_dram = dram.tile([E * SE + 1, D], f32)
        cc_in = dram.tile([9, 8], f32)
        cc_out = dram.tile([72, 8], f32, addr_space="Shared")

        const = top.enter_context(tc.tile_pool(name="const", bufs=1))
        ident = const.tile([128, 128], f32)
        make_identity(nc, ident[:])
        lincl = const.tile([128, 128], f32)
        nc.gpsimd.memset(lincl[:], 1.0)
        nc.gpsimd.affine_select(out=lincl[:], in_=lincl[:],
                                compare_op=ALU.is_ge, fill=0.0, base=0,
                                pattern=[[1, 128]], channel_multiplier=-1)
        lstrict = const.tile([128, 128], f32)
        nc.gpsimd.memset(lstrict[:], 1.0)
        nc.gpsimd.affine_select(out=lstrict[:], in_=lstrict[:],
                                compare_op=ALU.is_gt, fill=0.0, base=0,
                                pattern=[[1, 128]], channel_multiplier=-1)
        dm = []
        for d in range(4):
            dmf = const.tile([128, 512], f32, tag="dmf", name=f"dmf_{d}")
            nc.gpsimd.memset(dmf[:], 1.0)
            nc.gpsimd.affine_select(out=dmf[:], in_=dmf[:],
                                    compare_op=ALU.is_ge, fill=0.0,
                                    base=-d * 128, pattern=[[1, 512]],
                                    channel_multiplier=-1)
            dmb = const.tile([128, 512], bf16, tag=f"dmb_{d}", name=f"dmb_{d}")
            nc.vector.tensor_copy(dmb[:], dmf[:])
            dm.append(dmb)
        ones_r = const.tile([128, 1], f32)
        nc.vector.memset(ones_r[:], 1.0)
        ones1r = const.tile([1, 128], f32)
        nc.vector.memset(ones1r[:], 1.0)
        iota8i = const.tile([128, 8], i32)
        nc.gpsimd.iota(iota8i[:], pattern=[[1, 8]], base=0, channel_multiplier=0)
        iota8 = const.tile([128, 8], f32)
        nc.vector.tensor_copy(iota8[:], iota8i[:])
        iotase = const.tile([128, 8], f32)
        nc.vector.tensor_scalar_mul(iotase[:], iota8[:], float(SE))
        eps_col = const.tile([128, 1], f32)
        nc.vector.memset(eps_col[:], LN_EPS)
        rbias_bc = const.tile([128, 17], f32)
        rb1 = const.tile([1, 17], f32)
        nc.sync.dma_start(rb1[:], din["rbias"][:])
        nc.gpsimd.partition_broadcast(rbias_bc[:], rb1[:])
        wrout_sb = const.tile([128, DC, 17], f32)
        nc.sync.dma_start(wrout_sb[:],
                          din["wrout"].rearrange("(a p) n -> p a n", p=128))
        chsel_sb = const.tile([64, E], f32)
        nc.sync.dma_start(chsel_sb[:], din["chunksel"][:])

        x1_t, xn2_t = [], []
        g1_t, g2_t, ns_t, m0_t, m1_t, m_t, maug_t, gf_t = ([] for _ in range(8))
        keep_t, keepr_t, gidx_t = [], [], []

        with ExitStack() as sCF:
            if True:
                pbcp = sCF.enter_context(tc.tile_pool(name="pbcp", bufs=1))
                attT = pbcp.tile([128, DC, NOWN], f32r)
                xn2T = pbcp.tile([128, DC, NOWN], f32)
                with ExitStack() as sAB:
                    pab = sAB.enter_context(tc.tile_pool(name="pab", bufs=1))
                    xnT = pab.tile([128, DC, T], f32r)
                    vaug = pab.tile([128, NKC, H, 128], bf16)

                    # ---------- Phase A ----------
                    with ExitStack() as sA:
                        wvp = sA.enter_context(tc.tile_pool(name="wvp", bufs=1))
                        wv_sb = wvp.tile([128, DC, D], f32r)
                        nc.sync.dma_start(
                            wv_sb[:],
                            din["wv"].rearrange("(a p) n -> p a n",
                                                p=128).bitcast(f32r))
                        apool = sA.enter_context(tc.tile_pool(name="pa_sb",
                                                              bufs=2))
                        aps = sA.enter_context(
                            tc.tile_pool(name="pa_ps", bufs=2, space="PSUM"))
                        for kc in range(NKC):
                            xt = apool.tile([128, D], f32, tag="xt", name="xt")
                            nc.sync.dma_start(
                                xt[:], din["xp"][kc * 128:(kc + 1) * 128, :])
                            lsc = apool.tile([128, D], f32, tag="lsc",
                                             name="lsc")
                            xn = _ln(nc, apool, xt, eps_col, lsc)
                            tp4 = aps.tile([128, 512], f32, tag="tp",
                                           name="tp")
                            for i in range(4):
                                nc.tensor.transpose(
                                    tp4[:, i * 128:(i + 1) * 128],
                                    xn[:, i * 128:(i + 1) * 128], ident[:])
                            nc.vector.tensor_copy(
                                xnT[:, 0:4, kc * 128:(kc + 1) * 128],
                                tp4[:].rearrange("p (a n) -> p a n", a=4))
                            tp2 = aps.tile([128, 256], f32, tag="tp2b",
                                           name="tp2b")
                            for i in range(2):
                                nc.tensor.transpose(
                                    tp2[:, i * 128:(i + 1) * 128],
                                    xn[:, (4 + i) * 128:(5 + i) * 128],
                                    ident[:])
                            nc.vector.tensor_copy(
                                xnT[:, 4:6, kc * 128:(kc + 1) * 128],
                                tp2[:].rearrange("p (a n) -> p a n", a=2))
                            for nb in range(2):
                                vp = aps.tile([128, 384], f32, tag="vp",
                                              name="vp")
                                for dc in range(DC):
                                    nc.tensor.matmul(
                                        vp[:],
                                        xnT[:, dc, kc * 128:(kc + 1) * 128],
                                        wv_sb[:, dc, nb * 384:(nb + 1) * 384],
                                        start=(dc == 0), stop=(dc == DC - 1))
                                nc.vector.tensor_copy(
                                    vaug[:, kc, nb * 6:(nb + 1) * 6, 0:HS],
                                    vp[:].rearrange("p (h e) -> p h e", e=HS))
                        nc.gpsimd.memset(vaug[:, :, :, HS:128], 1.0)

                    # ---------- Phase B ----------
                    with ExitStack() as sB:
                        wqk = sB.enter_context(tc.tile_pool(name="wqk", bufs=1))
                        wq_sb = wqk.tile([128, DC, D], f32r, tag="wq_sb",
                                         name="wq_sb")
                        nc.sync.dma_start(
                            wq_sb[:],
                            din["wq"].rearrange("(a p) n -> p a n",
                                                p=128).bitcast(f32r))
                        wk_sb = wqk.tile([128, DC, D], f32r, tag="wk_sb",
                                         name="wk_sb")
                        nc.sync.dma_start(
                            wk_sb[:],
                            din["wk"].rearrange("(a p) n -> p a n",
                                                p=128).bitcast(f32r))
                        kqpool = sB.enter_context(tc.tile_pool(name="pb_kq",
                                                               bufs=1))
                        bpool = sB.enter_context(tc.tile_pool(name="pb_sb",
                                                              bufs=4))
                        recp = sB.enter_context(tc.tile_pool(name="pb_rec",
                                                             bufs=1))
                        bps = sB.enter_context(
                            tc.tile_pool(name="pb_ps", bufs=3, space="PSUM"))
                        atps = sB.enter_context(
                            tc.tile_pool(name="pb_at", bufs=2, space="PSUM"))
                        for h in range(H):
                            kT = kqpool.tile([66, T], f32r, tag="kT", name="kT")
                            nc.sync.dma_start(kT[64:66, :],
                                              din["kbias"][:].bitcast(f32r))
                            for qb in range(4):
                                kp = bps.tile([64, 512], f32, tag="kp",
                                              name="kp")
                                for dc in range(DC):
                                    nc.tensor.matmul(
                                        kp[:],
                                        wk_sb[:, dc, h * 64:(h + 1) * 64],
                                        xnT[:, dc, qb * 512:(qb + 1) * 512],
                                        start=(dc == 0), stop=(dc == DC - 1))
                                nc.vector.tensor_copy(
                                    kT[0:64, qb * 512:(qb + 1) * 512], kp[:])
                            qT = kqpool.tile([66, NOWN], f32r, tag="qT",
                                             name="qT")
                            nc.sync.dma_start(qT[64:66, :],
                                              din["qsel"][:].bitcast(f32r))
                            for qb in range(2):
                                qp = bps.tile([64, 512], f32, tag="kp",
                                              name="qp")
                                for dc in range(DC):
                                    nc.tensor.matmul(
                                        qp[:],
                                        wq_sb[:, dc, h * 64:(h + 1) * 64],
                                        xnT[:, dc, qb * 512:(qb + 1) * 512],
                                        start=(dc == 0), stop=(dc == DC - 1))
                                nc.vector.tensor_copy(
                                    qT[0:64, qb * 512:(qb + 1) * 512], qp[:])

                            for slot, kcs in ((0, KCS0), (1, KCS1)):
                                at = atps.tile([128, 512], f32, tag="at",
                                               name="at")
                                pend = []
                                for j, kc in enumerate(kcs):
                                    st = bps.tile([128, 512], f32, tag="st",
                                                  name="st")
                                    nc.tensor.matmul(
                                        st[:], kT[:, kc * 128:(kc + 1) * 128],
                                        qT[:, slot * 512:(slot + 1) * 512],
                                        start=True, stop=True)
                                    exr = bpool.tile([128, 512], bf16,
                                                     tag="exr", name="exr")
                                    nc.scalar.activation(exr[:], st[:], AF.Exp)
                                    if kc < 8 and kc // 4 == slot:
                                        nc.vector.tensor_tensor(
                                            out=exr[:], in0=exr[:],
                                            in1=dm[kc % 4][:], op=ALU.mult)
                                    pend.append((kc, exr))
                                    if len(pend) > 1:
                                        kcp, exrp = pend.pop(0)
                                        nc.tensor.matmul(
                                            at[:], vaug[:, kcp, h, :],
                                            exrp[:], start=(j == 1),
                                            stop=False)
                                kcp, exrp = pend.pop(0)
                                nc.tensor.matmul(
                                    at[:], vaug[:, kcp, h, :], exrp[:],
                                    start=False, stop=True)
                                rec = recp.tile([64, 512], f32, tag="rec",
                                                 name="rec")
                                nc.vector.reciprocal(rec[:], at[64:128, :])
                                nc.vector.tensor_tensor(
                                    out=attT[(h % 2) * 64:(h % 2) * 64 + 64,
                                             h // 2,
                                             slot * 512:(slot + 1) * 512],
                                    in0=at[0:64, :], in1=rec[:], op=ALU.mult)

                # ---------- Phase C ----------
                cpersist = top.enter_context(tc.tile_pool(name="cpersist",
                                                          bufs=1,
                                                          side="right"))
                with ExitStack() as sC:
                    wop = sC.enter_context(tc.tile_pool(name="wop", bufs=1))
                    wo_sb = wop.tile([128, DC, D], f32r)
                    nc.sync.dma_start(
                        wo_sb[:],
                        din["wo"].rearrange("(a p) n -> p a n",
                                            p=128).bitcast(f32r))
                    ctmp = sC.enter_context(tc.tile_pool(name="ctmp", bufs=2))
                    cps = sC.enter_context(
                        tc.tile_pool(name="pc_ps", bufs=2, space="PSUM"))
                    for tt in range(8):
                        xo = ctmp.tile([128, D], f32, tag="xo", name="xo")
                        nc.sync.dma_start(
                            xo[:], din["xp"][tt * 128:(tt + 1) * 128, :])
                        x1 = cpersist.tile([128, D], f32, tag=f"x1_{tt}",
                                           name=f"x1_{tt}")
                        for nb in range(2):
                            yp = cps.tile([128, 384], f32, tag="yp", name="yp")
                            for dc in range(DC):
                                nc.tensor.matmul(
                                    yp[:],
                                    attT[:, dc, tt * 128:(tt + 1) * 128],
                                    wo_sb[:, dc, nb * 384:(nb + 1) * 384],
                                    start=(dc == 0), stop=(dc == DC - 1))
                            nc.vector.tensor_add(
                                x1[:, nb * 384:(nb + 1) * 384], yp[:],
                                xo[:, nb * 384:(nb + 1) * 384])
                        nc.sync.dma_start(x1dbg[tt * 128:(tt + 1) * 128, :],
                                          x1[:])
                        xn2raw = _ln(nc, ctmp, x1, eps_col, xo)
                        xn2 = cpersist.tile([128, D], f32, tag=f"xn2_{tt}",
                                            name=f"xn2_{tt}")
                        nc.vector.tensor_copy(xn2[:], xn2raw[:])
                        tp4 = cps.tile([128, 512], f32, tag="tp2", name="tp2")
                        for i in range(4):
                            nc.tensor.transpose(
                                tp4[:, i * 128:(i + 1) * 128],
                                xn2[:, i * 128:(i + 1) * 128], ident[:])
                        nc.vector.tensor_copy(
                            xn2T[:, 0:4, tt * 128:(tt + 1) * 128],
                            tp4[:].rearrange("p (a n) -> p a n", a=4))
                        tp2 = cps.tile([128, 256], f32, tag="tp2c",
                                       name="tp2c")
                        for i in range(2):
                            nc.tensor.transpose(
                                tp2[:, i * 128:(i + 1) * 128],
                                xn2[:, (4 + i) * 128:(5 + i) * 128], ident[:])
                        nc.vector.tensor_copy(
                            xn2T[:, 4:6, tt * 128:(tt + 1) * 128],
                            tp2[:].rearrange("p (a n) -> p a n", a=2))
                        x1_t.append(x1)
                        xn2_t.append(xn2)

            # ---------- Phase D: router ----------
            rpool = top.enter_context(tc.tile_pool(name="rpool", bufs=1,
                                                   side="right"))
            rps = sCF.enter_context(tc.tile_pool(name="pd_ps", bufs=1,
                                                 space="PSUM"))
            cnt_ps = rps.tile([9, 8], f32)
            with ExitStack() as pd:
                dps = pd.enter_context(
                    tc.tile_pool(name="pd_ps2", bufs=2, space="PSUM"))
                dpool = pd.enter_context(tc.tile_pool(name="pd_tmp", bufs=2))
                dkeep = pd.enter_context(tc.tile_pool(name="pd_keep", bufs=1))
                rt_t, zp1_t, sp0_t, nt_t = [], [], [], []
                # pass 1: noise DMAs + router logits + Exp (one table)
                for tt in range(8):
                    nt = dkeep.tile([128, 8], f32, tag=f"nt_{tt}",
                                    name=f"nt_{tt}")
                    nc.sync.dma_start(
                        nt[:], din["noise"][tt * 128:(tt + 1) * 128, :])
                    nt_t.append(nt)
                for tt in range(8):
                    rp = dps.tile([128, 17], f32, tag="rp", name="rp")
                    for dc in range(DC):
                        nc.tensor.matmul(
                            rp[:], xn2T[:, dc, tt * 128:(tt + 1) * 128],
                            wrout_sb[:, dc, :],
                            start=(dc == 0), stop=(dc == DC - 1))
                    rt = dkeep.tile([128, 17], f32, tag=f"rt_{tt}",
                                    name=f"rt_{tt}")
                    nc.vector.tensor_add(rt[:], rp[:], rbias_bc[:])
                    z = dpool.tile([128, 8], f32, tag="z", name="z")
                    nc.scalar.activation(z[:], rt[:, 8:16], AF.Exp)
                    zp1 = dkeep.tile([128, 8], f32, tag=f"zp1_{tt}",
                                     name=f"zp1_{tt}")
                    nc.vector.tensor_scalar_add(zp1[:], z[:], 1.0)
                    rt_t.append(rt); zp1_t.append(zp1)
                # pass 2: Ln (one table)
                for tt in range(8):
                    sp0 = dkeep.tile([128, 8], f32, tag=f"sp0_{tt}",
                                     name=f"sp0_{tt}")
                    nc.scalar.activation(sp0[:], zp1_t[tt][:], AF.Ln)
                    sp0_t.append(sp0)
                # pass 3: Exp (one table) + vector chain
                for tt in range(8):
                    rt, zp1, sp0, nt = (rt_t[tt], zp1_t[tt], sp0_t[tt],
                                        nt_t[tt])
                    en = dpool.tile([128, 8], f32, tag="en", name="en")
                    nc.scalar.activation(en[:], sp0[:], AF.Exp, scale=-1.0)
                    t1 = dpool.tile([128, 8], f32, tag="t1", name="t1")
                    nc.vector.tensor_tensor(out=t1[:], in0=zp1[:], in1=en[:],
                                            op=ALU.mult)
                    nc.vector.tensor_scalar_add(t1[:], t1[:], -1.0)
                    sp = dpool.tile([128, 8], f32, tag="sp", name="sp")
                    nc.vector.tensor_add(sp[:], sp0[:], t1[:])
                    nm = dpool.tile([128, 8], f32, tag="nm", name="nm")
                    nc.vector.tensor_tensor(out=nm[:], in0=nt[:], in1=sp[:],
                                            op=ALU.mult)
                    noisy = dpool.tile([128, 8], f32, tag="noisy", name="noisy")
                    nc.vector.tensor_add(noisy[:], rt[:, 0:8], nm[:])
                    t8 = dpool.tile([128, 8], f32, tag="t8", name="t8")
                    nc.vector.max(t8[:], noisy[:])
                    ix = dpool.tile([128, 8], u32, tag="ix", name="ix")
                    nc.vector.max_index(ix[:], t8[:], noisy[:])
                    ixf = dpool.tile([128, 8], f32, tag="ixf", name="ixf")
                    nc.vector.tensor_copy(ixf[:], ix[:])
                    dv = dpool.tile([128, 1], f32, tag="dv", name="dv")
                    nc.vector.tensor_sub(dv[:], t8[:, 1:2], t8[:, 0:1])
                    ge = dpool.tile([128, 1], f32, tag="ge", name="ge")
                    nc.scalar.activation(ge[:], dv[:], AF.Exp)
                    gp1 = dpool.tile([128, 1], f32, tag="gp1", name="gp1")
                    nc.vector.tensor_scalar_add(gp1[:], ge[:], 1.0)
                    g1 = rpool.tile([128, 1], f32, tag=f"g1_{tt}",
                                    name=f"g1_{tt}")
                    nc.vector.reciprocal(g1[:], gp1[:])
                    g2 = rpool.tile([128, 1], f32, tag=f"g2_{tt}",
                                    name=f"g2_{tt}")
                    nc.vector.tensor_tensor(out=g2[:], in0=ge[:], in1=g1[:],
                                            op=ALU.mult)
                    ns = rpool.tile([128, 1], f32, tag=f"ns_{tt}",
                                    name=f"ns_{tt}")
                    nc.vector.tensor_scalar(ns[:], rt[:, 16:17], 0.0, None,
                                            ALU.is_le)
                    m0 = rpool.tile([128, 8], f32, tag=f"m0_{tt}",
                                    name=f"m0_{tt}")
                    nc.vector.tensor_scalar(m0[:], iota8[:], ixf[:, 0:1], None,
                                            ALU.is_equal)
                    m1 = rpool.tile([128, 8], f32, tag=f"m1_{tt}",
                                    name=f"m1_{tt}")
                    nc.vector.tensor_scalar(m1[:], iota8[:], ixf[:, 1:2], None,
                                            ALU.is_equal)
                    gf = rpool.tile([128, 8], f32, tag=f"gf_{tt}",
                                    name=f"gf_{tt}")
                    ga = dpool.tile([128, 8], f32, tag="ga", name="ga")
                    nc.vector.tensor_scalar(ga[:], m0[:], g1[:, :1], None,
                                            ALU.mult)
                    gb = dpool.tile([128, 8], f32, tag="gb", name="gb")
                    nc.vector.tensor_scalar(gb[:], m1[:], g2[:, :1], None,
                                            ALU.mult)
                    nc.vector.tensor_add(gf[:], ga[:], gb[:])
                    m = rpool.tile([128, 8], f32, tag=f"m_{tt}", name=f"m_{tt}")
                    nc.vector.tensor_add(m[:], m0[:], m1[:])
                    nc.vector.tensor_scalar_min(m[:], m[:], 1.0)
                    nc.vector.tensor_scalar(m[:], m[:], ns[:, :1], None,
                                            ALU.mult)
                    maug = rpool.tile([128, 9], f32, tag=f"maug_{tt}",
                                      name=f"maug_{tt}")
                    nc.vector.tensor_copy(maug[:, 0:8], m[:])
                    nc.vector.tensor_copy(maug[:, 8:9], ns[:])
                    nc.tensor.matmul(cnt_ps[:, tt:tt + 1], maug[:], ones_r[:],
                                     start=True, stop=True)
                    g1p = rpool.tile([128, 1], f32, tag=f"g1p_{tt}",
                                     name=f"g1p_{tt}")
                    nc.vector.tensor_tensor(out=g1p[:], in0=g1[:], in1=ns[:],
                                            op=ALU.mult)
                    g2p = rpool.tile([128, 1], f32, tag=f"g2p_{tt}",
                                     name=f"g2p_{tt}")
                    nc.vector.tensor_tensor(out=g2p[:], in0=g2[:], in1=ns[:],
                                            op=ALU.mult)
                    nsp = rpool.tile([128, 1], f32, tag=f"nsp_{tt}",
                                     name=f"nsp_{tt}")
                    nc.vector.tensor_scalar(nsp[:], ns[:], 1.0, -1.0,
                                            ALU.subtract, ALU.mult)
                    g1_t.append(g1p); g2_t.append(g2p); ns_t.append(nsp)
                    m0_t.append(m0); m1_t.append(m1); m_t.append(m)
                    maug_t.append(maug); gf_t.append(gf)

            cnt_sb = rpool.tile([9, 8], f32)
            nc.vector.tensor_copy(cnt_sb[:], cnt_ps[:])
            nc.sync.dma_start(cc_in[:], cnt_sb[:])
            nc.gpsimd.collective_compute(
                "AllGather", ALU.bypass, replica_groups=[list(range(NCORES))],
                ins=[cc_in.opt()], outs=[cc_out.opt()])
            cnts_all = rpool.tile([72, 8], f32)
            nc.sync.dma_start(cnts_all[:], cc_out[:])

            # ---------- Phase E ----------
            flat = rpool.tile([9, 64], f32)
            for r in range(NCORES):
                b2, a = r // 2, r % 2
                if a == 0:
                    nc.sync.dma_start(flat[:, b2 * 16:b2 * 16 + 4],
                                      cnts_all[9 * r:9 * r + 9, 0:4])
                    nc.sync.dma_start(flat[:, b2 * 16 + 12:b2 * 16 + 16],
                                      cnts_all[9 * r:9 * r + 9, 4:8])
                else:
                    nc.sync.dma_start(flat[:, b2 * 16 + 4:b2 * 16 + 12],
                                      cnts_all[9 * r:9 * r + 9, 0:8])
            zf = rpool.tile([9, 64], f32)
            nc.vector.memset(zf[:], 0.0)
            incl = rpool.tile([9, 64], f32)
            nc.vector.tensor_tensor_scan(incl[:], flat[:], zf[:], 0.0,
                                         ALU.add, ALU.add)
            excl = rpool.tile([9, 64], f32)
            nc.vector.tensor_sub(excl[:], incl[:], flat[:])
            tot = rpool.tile([1, 1], f32)
            nc.sync.dma_start(tot[:], incl[8:9, 63:64])
            tot_i = rpool.tile([1, 1], i32)
            nc.vector.tensor_copy(tot_i[:], tot[:])
            cap_i = rpool.tile([1, 1], i32)
            nc.vector.tensor_scalar(cap_i[:], tot_i[:], 2, None,
                                    ALU.arith_shift_right)
            capt = rpool.tile([1, 1], f32)
            nc.vector.tensor_copy(capt[:], cap_i[:])
            cap_bc = rpool.tile([128, 1], f32)
            nc.gpsimd.partition_broadcast(cap_bc[:], capt[:])

            exT_ps = rps.tile([64, 9], f32)
            nc.tensor.transpose(exT_ps[:], excl[:, 0:64], ident[0:9, 0:9])
            exT = rpool.tile([64, 9], f32)
            nc.vector.tensor_copy(exT[:], exT_ps[:])
            myo_ps = rps.tile([9, 8], f32)
            nc.tensor.matmul(myo_ps[:], exT[:, 0:9], chsel_sb[:], start=True,
                             stop=True)
            myo = rpool.tile([9, 8], f32)
            nc.vector.tensor_copy(myo[:], myo_ps[:])
            myoT_ps = rps.tile([8, 9], f32)
            nc.tensor.transpose(myoT_ps[:], myo[:], ident[0:9, 0:9])
            myoT = rpool.tile([8, 9], f32)
            nc.vector.tensor_copy(myoT[:], myoT_ps[:])

            # ---------- Phase F ----------
            kcnt_ps = rps.tile([8, 8], f32)
            with ExitStack() as pf:
                fps = pf.enter_context(
                    tc.tile_pool(name="pf_ps", bufs=2, space="PSUM"))
                for tt in range(8):
                    pr = fps.tile([128, 8], f32, tag="pr", name="pr")
                    orow = rpool.tile([1, 8], f32, tag=f"orow_{tt}",
                                      name=f"orow_{tt}")
                    nc.sync.dma_start(orow[:], myoT[tt:tt + 1, 0:8])
                    nc.tensor.matmul(pr[:], ones1r[:], orow[:],
                                     start=True, stop=False)
                    nc.tensor.matmul(pr[:], lincl[:], maug_t[tt][:, 0:8],
                                     start=False, stop=True)
                    keepb = rpool.tile([128, 8], f32, tag=f"kb_{tt}",
                                       name=f"kb_{tt}")
                    nc.vector.tensor_scalar(keepb[:], pr[:], cap_bc[:, :1],
                                            None, ALU.is_le)
                    keep = rpool.tile([128, 8], f32, tag=f"keep_{tt}",
                                      name=f"keep_{tt}")
                    nc.vector.tensor_tensor(out=keep[:], in0=keepb[:],
                                            in1=m_t[tt][:], op=ALU.mult)
                    nc.tensor.matmul(kcnt_ps[:, tt:tt + 1], keep[:],
                                     ones_r[:], start=True, stop=True)
                    kg = rpool.tile([128, 8], f32, tag=f"kg_{tt}",
                                    name=f"kg_{tt}")
                    nc.vector.tensor_tensor(out=kg[:], in0=keep[:],
                                            in1=gf_t[tt][:], op=ALU.mult)
                    nc.sync.dma_start(rdbg[tt * 128:(tt + 1) * 128, :], kg[:])
                    keep_t.append(keep); keepr_t.append(keep)

            kcnt = rpool.tile([8, 8], f32)
            nc.vector.tensor_copy(kcnt[:], kcnt_ps[:])
            zf8 = rpool.tile([8, 8], f32)
            nc.vector.memset(zf8[:], 0.0)
            kincl = rpool.tile([8, 8], f32)
            nc.vector.tensor_tensor_scan(kincl[:], kcnt[:], zf8[:], 0.0,
                                         ALU.add, ALU.add)
            kexcl = rpool.tile([8, 8], f32)
            nc.vector.tensor_sub(kexcl[:], kincl[:], kcnt[:])
            kexT_ps = rps.tile([8, 8], f32)
            nc.tensor.transpose(kexT_ps[:], kexcl[:], ident[0:8, 0:8])
            kexT = rpool.tile([8, 8], f32)
            nc.vector.tensor_copy(kexT[:], kexT_ps[:])

            with ExitStack() as pf2:
                f2ps = pf2.enter_context(
                    tc.tile_pool(name="pf2_ps", bufs=2, space="PSUM"))
                f2p = pf2.enter_context(tc.tile_pool(name="pf2_sb", bufs=2))
                for tt in range(8):
                    p2 = f2ps.tile([128, 8], f32, tag="p2", name="p2")
                    krow = rpool.tile([1, 8], f32, tag=f"krow_{tt}",
                                      name=f"krow_{tt}")
                    nc.sync.dma_start(krow[:], kexT[tt:tt + 1, :])
                    nc.tensor.matmul(p2[:], ones1r[:], krow[:],
                                     start=True, stop=False)
                    nc.tensor.matmul(p2[:], lstrict[:], keepr_t[tt][:],
                                     start=False, stop=True)
                    slotf = f2p.tile([128, 8], f32, tag="slotf", name="slotf")
                    nc.vector.tensor_add(slotf[:], p2[:], iotase[:])
                    gidx = rpool.tile([128, 2], i32, tag=f"gi_{tt}",
                                      name=f"gi_{tt}")
                    for k, mk in ((0, m0_t[tt]), (1, m1_t[tt])):
                        fim = f2p.tile([128, 8], f32, tag="fim", name="fim")
                        nc.vector.tensor_tensor(out=fim[:], in0=slotf[:],
                                                in1=mk[:], op=ALU.mult)
                        fi = f2p.tile([128, 1], f32, tag="fi", name="fi")
                        nc.vector.reduce_sum(fi[:], fim[:],
                                             axis=mybir.AxisListType.X)
                        km = f2p.tile([128, 8], f32, tag="km", name="km")
                        nc.vector.tensor_tensor(out=km[:], in0=mk[:],
                                                in1=keep_t[tt][:], op=ALU.mult)
                        kept = f2p.tile([128, 1], f32, tag="kept", name="kept")
                        nc.vector.reduce_sum(kept[:], km[:],
                                             axis=mybir.AxisListType.X)
                        u = f2p.tile([128, 1], f32, tag="u", name="u")
                        nc.vector.tensor_scalar_add(u[:], kept[:], -1.0)
                        nc.vector.tensor_scalar_mul(u[:], u[:], -70000.0)
                        fis = f2p.tile([128, 1], f32, tag="fis", name="fis")
                        nc.vector.tensor_add(fis[:], fi[:], u[:])
                        fii = f2p.tile([128, 1], i32, tag="fii", name="fii")
                        nc.vector.tensor_copy(fii[:], fis[:])
                        nc.gpsimd.indirect_dma_start(
                            out=xe_dram[:],
                            out_offset=bass.IndirectOffsetOnAxis(
                                ap=fii[:, :1], axis=0),
                            in_=xn2_t[tt][:], in_offset=None,
                            bounds_check=E * SE - 1, oob_is_err=False)
                        gi = f2p.tile([128, 1], f32, tag="gi2", name="gi2")
                        nc.vector.tensor_tensor(out=gi[:], in0=fi[:],
                                                in1=kept[:], op=ALU.mult)
                        w = f2p.tile([128, 1], f32, tag="u2", name="u2")
                        nc.vector.tensor_scalar_add(w[:], kept[:], -1.0)
                        nc.vector.tensor_scalar_mul(w[:], w[:], -float(ZROW))
                        nc.vector.tensor_add(gi[:], gi[:], w[:])
                        nc.vector.tensor_copy(gidx[:, k:k + 1], gi[:])
                    gidx_t.append(gidx)

        # ---------- Phase G: expert MLPs ----------
        zrow = rpool.tile([128, D], f32)
        nc.vector.memset(zrow[:], 0.0)
        nc.sync.dma_start(ye_dram[ZROW:ZROW + 1, :], zrow[0:1, :])
        with ExitStack() as pg:
            gsb = pg.enter_context(tc.tile_pool(name="pg_sb", bufs=2))
            xetp = pg.enter_context(tc.tile_pool(name="pg_xet", bufs=1))
            w1p = pg.enter_context(tc.tile_pool(name="pg_w1", bufs=6))
            w2p = pg.enter_context(tc.tile_pool(name="pg_w2", bufs=6))
            hpool = pg.enter_context(tc.tile_pool(name="pg_h", bufs=2))
            ROWS = [(0, 128), (128, 64)]
            # all-expert activation transposes upfront (dense PE burst,
            # removes per-expert boundary stalls)
            xet = xetp.tile([128, E, DC, SE], f8)
            with ExitStack() as pgt:
                tps = pgt.enter_context(
                    tc.tile_pool(name="pg_tps", bufs=2, space="PSUM"))
                for e in range(E):
                    for r0, rn in ROWS:
                        xe = gsb.tile([128, D], f32, tag="xe", name="xe")
                        nc.sync.dma_start(
                            xe[0:rn, :],
                            xe_dram[e * SE + r0:e * SE + r0 + rn, :])
                        tp4 = tps.tile([128, 512], f32, tag="tp3", name="tp3")
                        for i in range(4):
                            nc.tensor.transpose(
                                tp4[:, i * 128:i * 128 + rn],
                                xe[0:rn, i * 128:(i + 1) * 128],
                                ident[0:rn, 0:rn])
                        nc.vector.tensor_scalar_mul(
                            xet[:, e, 0:4, r0:r0 + rn],
                            tp4[:].rearrange("p (a n) -> p a n",
                                             a=4)[:, :, 0:rn],
                            XSCALE)
                        tp2 = tps.tile([128, 256], f32, tag="tp3b",
                                       name="tp3b")
                        for i in range(2):
                            nc.tensor.transpose(
                                tp2[:, i * 128:i * 128 + rn],
                                xe[0:rn, (4 + i) * 128:(5 + i) * 128],
                                ident[0:rn, 0:rn])
                        nc.vector.tensor_scalar_mul(
                            xet[:, e, 4:6, r0:r0 + rn],
                            tp2[:].rearrange("p (a n) -> p a n",
                                             a=2)[:, :, 0:rn],
                            XSCALE)
            gps = pg.enter_context(
                tc.tile_pool(name="pg_ps", bufs=2, space="PSUM"))
            yps = pg.enter_context(
                tc.tile_pool(name="pg_yps", bufs=1, space="PSUM"))
            for e in range(E):
                hT = hpool.tile([128, 24, SE], f8, tag="hT", name="hT")
                for q in range(6):
                    w1t = w1p.tile([128, 4, DC, 128], f8, tag="w1t",
                                   name="w1t")
                    nc.sync.dma_start(
                        w1t[:],
                        din["we1"][e, q].rearrange("p (i a n) -> p i a n",
                                                   i=4, a=DC))
                    for i in range(4):
                        mt = 4 * q + i
                        hp = gps.tile([128, SE], f32, tag="hp", name="hp")
                        for dc in range(DC):
                            nc.tensor.matmul(hp[:], w1t[:, i, dc, :],
                                             xet[:, e, dc, :],
                                             start=(dc == 0),
                                             stop=(dc == DC - 1))
                        # relu + rescale (x16/w64 -> keep x16 on h) + fp8 cast
                        nc.vector.tensor_scalar(hT[:, mt, :], hp[:], 0.0,
                                                1.0 / WSCALE, ALU.max,
                                                ALU.mult)
                ypl = [yps.tile([128, 384], f32, tag=f"yp_{i}",
                                name=f"ypl_{i}") for i in range(4)]
                for q in range(6):
                    w2t = w2p.tile([128, 4, D], f8, tag="w2t", name="w2t")
                    nc.sync.dma_start(
                        w2t[:],
                        din["we2"][e, q].rearrange("p (i n) -> p i n", i=4))
                    for i in range(4):
                        hc = 4 * q + i
                        for rt2, (r0, rn) in enumerate(ROWS):
                            for nb in range(2):
                                nc.tensor.matmul(
                                    ypl[rt2 * 2 + nb][0:rn, :],
                                    hT[:, hc, r0:r0 + rn],
                                    w2t[:, i, nb * 384:(nb + 1) * 384],
                                    start=(hc == 0), stop=(hc == 23))
                for rt2, (r0, rn) in enumerate(ROWS):
                    ysb = gsb.tile([128, D], f32, tag="ysb", name="ysb")
                    for nb in range(2):
                        nc.vector.tensor_scalar_mul(
                            ysb[0:rn, nb * 384:(nb + 1) * 384],
                            ypl[rt2 * 2 + nb][0:rn, :],
                            1.0 / (XSCALE * WSCALE))
                    nc.sync.dma_start(
                        ye_dram[e * SE + r0:e * SE + r0 + rn, :],
                        ysb[0:rn, :])

        # ---------- Phase H ----------
        with ExitStack() as ph:
            hsb = ph.enter_context(tc.tile_pool(name="ph_sb", bufs=3))
            for tt in range(8):
                yg0 = hsb.tile([128, D], f32, tag="yg0", name="yg0")
                nc.gpsimd.indirect_dma_start(
                    out=yg0[:], out_offset=None, in_=ye_dram[:],
                    in_offset=bass.IndirectOffsetOnAxis(
                        ap=gidx_t[tt][:, 0:1], axis=0))
                yg1 = hsb.tile([128, D], f32, tag="yg1", name="yg1")
                nc.gpsimd.indirect_dma_start(
                    out=yg1[:], out_offset=None, in_=ye_dram[:],
                    in_offset=bass.IndirectOffsetOnAxis(
                        ap=gidx_t[tt][:, 1:2], axis=0))
                # out = x1 + (ns*g1)*yg0 + (ns*g2)*yg1 + (1-ns)*xn2
                u0 = hsb.tile([128, D], f32, tag="u0", name="u0")
                nc.vector.tensor_scalar(u0[:], yg0[:], g1_t[tt][:, :1], None,
                                        ALU.mult)
                u1 = hsb.tile([128, D], f32, tag="u1", name="u1")
                nc.scalar.activation(u1[:], yg1[:], AF.Copy,
                                     scale=g2_t[tt][:, :1])
                w = hsb.tile([128, D], f32, tag="w", name="w")
                nc.vector.tensor_scalar(w[:], xn2_t[tt][:], ns_t[tt][:, :1],
                                        None, ALU.mult)
                s01 = hsb.tile([128, D], f32, tag="s01", name="s01")
                nc.vector.tensor_add(s01[:], u0[:], u1[:])
                wx = hsb.tile([128, D], f32, tag="wx", name="wx")
                nc.vector.tensor_add(wx[:], w[:], x1_t[tt][:])
                out = hsb.tile([128, D], f32, tag="out", name="out")
                nc.vector.tensor_add(out[:], s01[:], wx[:])
                nc.sync.dma_start(yout[tt * 128:(tt + 1) * 128, :], out[:])

    nc.compile()
    return nc


_OWN = {0: [0, 1, 2, 3, 12, 13, 14, 15], 1: [4, 5, 6, 7, 8, 9, 10, 11]}


def _core_meta(c):
    b, a = c // 2, c % 2
    own = _OWN[a]
    other = [g for g in range(16) if g not in own]
    perm_chunks = own + other
    rows = np.concatenate([np.arange(g * 128, (g + 1) * 128)
                           for g in perm_chunks])
    return b, a, own, rows


def _host_inputs(x, noise, Wq, Wk, Wv, Wo, Wr, br, Wn, bn, Wsk, bsk, We1, We2,
                 **_unused):
    x = np.asarray(x, np.float32)
    noise = np.asarray(noise, np.float32)
    wq = np.ascontiguousarray(
        (np.transpose(np.asarray(Wq), (1, 0, 2)).reshape(D, D)
         * np.float32(D ** -0.5)).astype(np.float32))
    wk = np.ascontiguousarray(
        np.transpose(np.asarray(Wk), (1, 0, 2)).reshape(D, D)
        .astype(np.float32))
    wv = np.ascontiguousarray(
        np.transpose(np.asarray(Wv), (1, 0, 2)).reshape(D, D)
        .astype(np.float32))
    wrout = np.ascontiguousarray(np.concatenate(
        [np.asarray(Wr), np.asarray(Wn), np.asarray(Wsk)], axis=1)
        .astype(np.float32))
    rbias = np.concatenate(
        [np.asarray(br), np.asarray(bn), np.asarray(bsk)])[None, :] \
        .astype(np.float32)
    qsel = np.zeros((2, NOWN), np.float32)
    qsel[0, 0:512] = 1.0
    qsel[1, 512:1024] = 1.0
    import ml_dtypes
    # we1[e, q, p, (i, a, n)] = We1[e, a*128+p, (4q+i)*128+n] * WSCALE  (fp8)
    W1 = (np.asarray(We1, np.float32) * WSCALE).reshape(E, DC, 128, 6, 4, 128)
    we1 = np.ascontiguousarray(
        W1.transpose(0, 3, 2, 4, 1, 5).reshape(E, 6, 128, 3072)
        .astype(ml_dtypes.float8_e4m3))
    # we2[e, q, p, (i, n)] = We2[e, (4q+i)*128+p, n] * WSCALE  (fp8)
    W2 = (np.asarray(We2, np.float32) * WSCALE).reshape(E, 6, 4, 128, D)
    we2 = np.ascontiguousarray(
        W2.transpose(0, 1, 3, 2, 4).reshape(E, 6, 128, 3072)
        .astype(ml_dtypes.float8_e4m3))
    wo = np.ascontiguousarray(np.asarray(Wo, np.float32))

    in_maps = []
    for c in range(NCORES):
        b, a, own, rows = _core_meta(c)
        gid = rows
        kbias = np.zeros((2, T), np.float32)
        for s in range(2):
            qmax = gid[s * 512:(s + 1) * 512].max()
            kbias[s] = np.where(gid > qmax, NEG, 0.0).astype(np.float32)
        chunksel = np.zeros((64, E), np.float32)
        for lc in range(8):
            chunksel[b * 16 + own[lc], lc] = 1.0
        in_maps.append({
            "xp": np.ascontiguousarray(x[b][rows]),
            "noise": np.ascontiguousarray(noise[b][rows[:NOWN]]),
            "wq": wq, "wk": wk, "wv": wv, "wo": wo,
            "wrout": wrout, "rbias": rbias,
            "we1": we1, "we2": we2,
            "kbias": kbias, "qsel": qsel,
            "chunksel": chunksel,
        })
    return in_maps


_prog = None


def run(trace=False, **inputs):
    global _prog
    if _prog is None:
        _prog = build_program()
    in_maps = _host_inputs(**inputs)
    res = bass_utils.run_bass_kernel_spmd(
        _prog, in_maps, core_ids=list(range(NCORES)), trace=trace)
    out = np.zeros((B, T, D), np.float32)
    for c in range(NCORES):
        b, a, own, rows = _core_meta(c)
        out[b][rows[:NOWN]] = res.results[c]["yout"]
    return out, res


def kernel(**inputs):
    out, _ = run(trace=False, **inputs)
    return out



# revision 25
# speedup vs baseline: 1.1014x; 1.1014x over previous
"""CrossLayerBlock kernel: baseline + fp8-DoubleRow expert MLPs."""
import numpy as np
from contextlib import ExitStack

import concourse.bass as bass
import concourse.tile as tile
from concourse import bacc, mybir
from concourse import bass_utils
from concourse.masks import make_identity

B, T, D, H, HS, E = 4, 2048, 768, 12, 64, 8
NCORES = 8
NOWN = 1024
NKC = 16
DC = 6
SE = 192
ZROW = E * SE
LN_EPS = 1e-5
NEG = -30.0

f32 = mybir.dt.float32
f32r = mybir.dt.float32r
bf16 = mybir.dt.bfloat16
f8 = mybir.dt.float8e4
WSCALE = 64.0
XSCALE = 16.0
i32 = mybir.dt.int32
u32 = mybir.dt.uint32
AF = mybir.ActivationFunctionType
ALU = mybir.AluOpType
DR = mybir.MatmulPerfMode.DoubleRow

KCS0 = [8, 9, 10, 11, 0, 1, 2, 3]
KCS1 = [8, 9, 10, 11, 12, 13, 14, 15, 0, 1, 2, 3, 4, 5, 6, 7]


def _ln(nc, pool, xt, eps_col, scratch):
    r = pool.tile([128, 1], f32, tag="ln_r", name="ln_r")
    nc.vector.reduce_sum(r[:], xt[:], axis=mybir.AxisListType.X)
    sq = scratch
    r2 = pool.tile([128, 1], f32, tag="ln_r2", name="ln_r2")
    nc.gpsimd.tensor_tensor(out=sq[:], in0=xt[:], in1=xt[:], op=ALU.mult)
    nc.vector.reduce_sum(r2[:], sq[:], axis=mybir.AxisListType.X)
    mu = pool.tile([128, 1], f32, tag="ln_mu", name="ln_mu")
    nc.vector.tensor_scalar_mul(mu[:], r[:], 1.0 / D)
    mr = pool.tile([128, 1], f32, tag="ln_mr", name="ln_mr")
    nc.vector.tensor_tensor(out=mr[:], in0=mu[:], in1=r[:], op=ALU.mult)
    vd = pool.tile([128, 1], f32, tag="ln_vd", name="ln_vd")
    nc.vector.tensor_sub(vd[:], r2[:], mr[:])
    sd = pool.tile([128, 1], f32, tag="ln_sd", name="ln_sd")
    nc.scalar.activation(sd[:], vd[:], AF.Sqrt, bias=eps_col[:, :1],
                         scale=1.0 / D)
    rstd = pool.tile([128, 1], f32, tag="ln_rstd", name="ln_rstd")
    nc.vector.reciprocal(rstd[:], sd[:])
    xn = pool.tile([128, D], f32, tag="ln_xn", name="ln_xn")
    nc.vector.tensor_scalar(xn[:], xt[:], mu[:, :1], rstd[:, :1],
                            ALU.subtract, ALU.mult)
    return xn


def build_program():
    nc = bacc.Bacc("TRN2", target_bir_lowering=False, debug=False,
                   enable_asserts=False, num_devices=NCORES)

    din = {}
    for name, shape, dt in [
        ("xp", [T, D], f32), ("noise", [NOWN, E], f32),
        ("wq", [D, D], f32), ("wk", [D, D], f32), ("wv", [D, D], f32),
        ("wo", [D, D], f32), ("wrout", [D, 17], f32), ("rbias", [1, 17], f32),
        ("we1", [E, 6, 128, 3072], f8), ("we2", [E, 6, 128, 3072], f8),
        ("kbias", [2, T], f32), ("qsel", [2, NOWN], f32),
        ("chunksel", [64, E], f32),
    ]:
        din[name] = nc.dram_tensor(name, shape, dt, kind="ExternalInput").ap()

    yout = nc.dram_tensor("yout", [NOWN, D], f32, kind="ExternalOutput").ap()
    x1dbg = nc.dram_tensor("x1dbg", [NOWN, D], f32, kind="ExternalOutput").ap()
    rdbg = nc.dram_tensor("rdbg", [NOWN, E], f32, kind="ExternalOutput").ap()

    with tile.TileContext(nc) as tc, ExitStack() as top:
        dram = top.enter_context(tc.tile_pool(name="dram", bufs=1, space="DRAM"))
        xe_dram = dram.tile([E * SE, D], f32)
        ye_dram = dram.tile([E * SE + 1, D], f32)
        cc_in = dram.tile([9, 8], f32)
        cc_out = dram.tile([72, 8], f32, addr_space="Shared")

        const = top.enter_context(tc.tile_pool(name="const", bufs=1))
        ident = const.tile([128, 128], f32)
        make_identity(nc, ident[:])
        lincl = const.tile([128, 128], f32)
        nc.gpsimd.memset(lincl[:], 1.0)
        nc.gpsimd.affine_select(out=lincl[:], in_=lincl[:],
                                compare_op=ALU.is_ge, fill=0.0, base=0,
                                pattern=[[1, 128]], channel_multiplier=-1)
        lstrict = const.tile([128, 128], f32)
        nc.gpsimd.memset(lstrict[:], 1.0)
        nc.gpsimd.affine_select(out=lstrict[:], in_=lstrict[:],
                                compare_op=ALU.is_gt, fill=0.0, base=0,
                                pattern=[[1, 128]], channel_multiplier=-1)
        dm = []
        for d in range(4):
            dmf = const.tile([128, 512], f32, tag="dmf", name=f"dmf_{d}")
            nc.gpsimd.memset(dmf[:], 1.0)
            nc.gpsimd.affine_select(out=dmf[:], in_=dmf[:],
                                    compare_op=ALU.is_ge, fill=0.0,
                                    base=-d * 128, pattern=[[1, 512]],
                                    channel_multiplier=-1)
            dmb = const.tile([128, 512], bf16, tag=f"dmb_{d}", name=f"dmb_{d}")
            nc.vector.tensor_copy(dmb[:], dmf[:])
            dm.append(dmb)
        ones_r = const.tile([128, 1], f32)
        nc.vector.memset(ones_r[:], 1.0)
        ones1r = const.tile([1, 128], f32)
        nc.vector.memset(ones1r[:], 1.0)
        iota8i = const.tile([128, 8], i32)
        nc.gpsimd.iota(iota8i[:], pattern=[[1, 8]], base=0, channel_multiplier=0)
        iota8 = const.tile([128, 8], f32)
        nc.vector.tensor_copy(iota8[:], iota8i[:])
        iotase = const.tile([128, 8], f32)
        nc.vector.tensor_scalar_mul(iotase[:], iota8[:], float(SE))
        eps_col = const.tile([128, 1], f32)
        nc.vector.memset(eps_col[:], LN_EPS)
        rbias_bc = const.tile([128, 17], f32)
        rb1 = const.tile([1, 17], f32)
        nc.sync.dma_start(rb1[:], din["rbias"][:])
        nc.gpsimd.partition_broadcast(rbias_bc[:], rb1[:])
        wrout_sb = const.tile([128, DC, 17], f32)
        nc.sync.dma_start(wrout_sb[:],
                          din["wrout"].rearrange("(a p) n -> p a n", p=128))
        chsel_sb = const.tile([64, E], f32)
        nc.sync.dma_start(chsel_sb[:], din["chunksel"][:])

        x1_t, xn2_t = [], []
        g1_t, g2_t, ns_t, m0_t, m1_t, m_t, maug_t, gf_t = ([] for _ in range(8))
        keep_t, keepr_t, gidx_t = [], [], []

        with ExitStack() as sCF:
            if True:
                pbcp = sCF.enter_context(tc.tile_pool(name="pbcp", bufs=1))
                attT = pbcp.tile([128, DC, NOWN], f32r)
                xn2T = pbcp.tile([128, DC, NOWN], f32)
                with ExitStack() as sAB:
                    pab = sAB.enter_context(tc.tile_pool(name="pab", bufs=1))
                    xnT = pab.tile([128, DC, T], f32r)
                    vaug = pab.tile([128, NKC, H, 128], bf16)

                    # ---------- Phase A ----------
                    with ExitStack() as sA:
                        wvp = sA.enter_context(tc.tile_pool(name="wvp", bufs=1))
                        wv_sb = wvp.tile([128, DC, D], f32r)
                        nc.sync.dma_start(
                            wv_sb[:],
                            din["wv"].rearrange("(a p) n -> p a n",
                                                p=128).bitcast(f32r))
                        apool = sA.enter_context(tc.tile_pool(name="pa_sb",
                                                              bufs=2))
                        aps = sA.enter_context(
                            tc.tile_pool(name="pa_ps", bufs=2, space="PSUM"))
                        for kc in range(NKC):
                            xt = apool.tile([128, D], f32, tag="xt", name="xt")
                            nc.sync.dma_start(
                                xt[:], din["xp"][kc * 128:(kc + 1) * 128, :])
                            lsc = apool.tile([128, D], f32, tag="lsc",
                                             name="lsc")
                            xn = _ln(nc, apool, xt, eps_col, lsc)
                            tp4 = aps.tile([128, 512], f32, tag="tp",
                                           name="tp")
                            for i in range(4):
                                nc.tensor.transpose(
                                    tp4[:, i * 128:(i + 1) * 128],
                                    xn[:, i * 128:(i + 1) * 128], ident[:])
                            nc.vector.tensor_copy(
                                xnT[:, 0:4, kc * 128:(kc + 1) * 128],
                                tp4[:].rearrange("p (a n) -> p a n", a=4))
                            tp2 = aps.tile([128, 256], f32, tag="tp2b",
                                           name="tp2b")
                            for i in range(2):
                                nc.tensor.transpose(
                                    tp2[:, i * 128:(i + 1) * 128],
                                    xn[:, (4 + i) * 128:(5 + i) * 128],
                                    ident[:])
                            nc.vector.tensor_copy(
                                xnT[:, 4:6, kc * 128:(kc + 1) * 128],
                                tp2[:].rearrange("p (a n) -> p a n", a=2))
                            for nb in range(2):
                                vp = aps.tile([128, 384], f32, tag="vp",
                                              name="vp")
                                for dc in range(DC):
                                    nc.tensor.matmul(
                                        vp[:],
                                        xnT[:, dc, kc * 128:(kc + 1) * 128],
                                        wv_sb[:, dc, nb * 384:(nb + 1) * 384],
                                        start=(dc == 0), stop=(dc == DC - 1))
                                nc.vector.tensor_copy(
                                    vaug[:, kc, nb * 6:(nb + 1) * 6, 0:HS],
                                    vp[:].rearrange("p (h e) -> p h e", e=HS))
                        nc.gpsimd.memset(vaug[:, :, :, HS:128], 1.0)

                    # ---------- Phase B ----------
                    with ExitStack() as sB:
                        wqk = sB.enter_context(tc.tile_pool(name="wqk", bufs=1))
                        wq_sb = wqk.tile([128, DC, D], f32r, tag="wq_sb",
                                         name="wq_sb")
                        nc.sync.dma_start(
                            wq_sb[:],
                            din["wq"].rearrange("(a p) n -> p a n",
                                                p=128).bitcast(f32r))
                        wk_sb = wqk.tile([128, DC, D], f32r, tag="wk_sb",
                                         name="wk_sb")
                        nc.sync.dma_start(
                            wk_sb[:],
                            din["wk"].rearrange("(a p) n -> p a n",
                                                p=128).bitcast(f32r))
                        kqpool = sB.enter_context(tc.tile_pool(name="pb_kq",
                                                               bufs=1))
                        bpool = sB.enter_context(tc.tile_pool(name="pb_sb",
                                                              bufs=4))
                        recp = sB.enter_context(tc.tile_pool(name="pb_rec",
                                                             bufs=1))
                        bps = sB.enter_context(
                            tc.tile_pool(name="pb_ps", bufs=3, space="PSUM"))
                        atps = sB.enter_context(
                            tc.tile_pool(name="pb_at", bufs=2, space="PSUM"))
                        for h in range(H):
                            kT = kqpool.tile([66, T], f32r, tag="kT", name="kT")
                            nc.sync.dma_start(kT[64:66, :],
                                              din["kbias"][:].bitcast(f32r))
                            for qb in range(4):
                                kp = bps.tile([64, 512], f32, tag="kp",
                                              name="kp")
                                for dc in range(DC):
                                    nc.tensor.matmul(
                                        kp[:],
                                        wk_sb[:, dc, h * 64:(h + 1) * 64],
                                        xnT[:, dc, qb * 512:(qb + 1) * 512],
                                        start=(dc == 0), stop=(dc == DC - 1))
                                nc.vector.tensor_copy(
                                    kT[0:64, qb * 512:(qb + 1) * 512], kp[:])
                            qT = kqpool.tile([66, NOWN], f32r, tag="qT",
                                             name="qT")
                            nc.sync.dma_start(qT[64:66, :],
                                              din["qsel"][:].bitcast(f32r))
                            for qb in range(2):
                                qp = bps.tile([64, 512], f32, tag="kp",
                                              name="qp")
                                for dc in range(DC):
                                    nc.tensor.matmul(
                                        qp[:],
                                        wq_sb[:, dc, h * 64:(h + 1) * 64],
                                        xnT[:, dc, qb * 512:(qb + 1) * 512],
                                        start=(dc == 0), stop=(dc == DC - 1))
                                nc.vector.tensor_copy(
                                    qT[0:64, qb * 512:(qb + 1) * 512], qp[:])

                            for slot, kcs in ((0, KCS0), (1, KCS1)):
                                at = atps.tile([128, 512], f32, tag="at",
                                               name="at")
                                pend = []
                                for j, kc in enumerate(kcs):
                                    st = bps.tile([128, 512], f32, tag="st",
                                                  name="st")
                                    nc.tensor.matmul(
                                        st[:], kT[:, kc * 128:(kc + 1) * 128],
                                        qT[:, slot * 512:(slot + 1) * 512],
                                        start=True, stop=True)
                                    exr = bpool.tile([128, 512], bf16,
                                                     tag="exr", name="exr")
                                    nc.scalar.activation(exr[:], st[:], AF.Exp)
                                    if kc < 8 and kc // 4 == slot:
                                        nc.vector.tensor_tensor(
                                            out=exr[:], in0=exr[:],
                                            in1=dm[kc % 4][:], op=ALU.mult)
                                    pend.append((kc, exr))
                                    if len(pend) > 1:
                                        kcp, exrp = pend.pop(0)
                                        nc.tensor.matmul(
                                            at[:], vaug[:, kcp, h, :],
                                            exrp[:], start=(j == 1),
                                            stop=False)
                                kcp, exrp = pend.pop(0)
                                nc.tensor.matmul(
                                    at[:], vaug[:, kcp, h, :], exrp[:],
                                    start=False, stop=True)
                                rec = recp.tile([64, 512], f32, tag="rec",
                                                 name="rec")
                                nc.vector.reciprocal(rec[:], at[64:128, :])
                                nc.vector.tensor_tensor(
                                    out=attT[(h % 2) * 64:(h % 2) * 64 + 64,
                                             h // 2,
                                             slot * 512:(slot + 1) * 512],
                                    in0=at[0:64, :], in1=rec[:], op=ALU.mult)

                # ---------- Phase C ----------
                cpersist = top.enter_context(tc.tile_pool(name="cpersist",
                                                          bufs=1,
                                                          side="right"))
                wpre = top.enter_context(tc.tile_pool(name="wpre", bufs=1,
                                                      side="right"))
                w1e0, w2e0 = [], []
                for q in range(6):
                    t1 = wpre.tile([128, 4, DC, 128], f8, tag=f"w1e0_{q}",
                                   name=f"w1e0_{q}")
                    nc.sync.dma_start(
                        t1[:],
                        din["we1"][0, q].rearrange("p (i a n) -> p i a n",
                                                   i=4, a=DC))
                    w1e0.append(t1)
                    t2 = wpre.tile([128, 4, D], f8, tag=f"w2e0_{q}",
                                   name=f"w2e0_{q}")
                    nc.sync.dma_start(
                        t2[:],
                        din["we2"][0, q].rearrange("p (i n) -> p i n", i=4))
                    w2e0.append(t2)
                with ExitStack() as sC:
                    wop = sC.enter_context(tc.tile_pool(name="wop", bufs=1))
                    wo_sb = wop.tile([128, DC, D], f32r)
                    nc.sync.dma_start(
                        wo_sb[:],
                        din["wo"].rearrange("(a p) n -> p a n",
                                            p=128).bitcast(f32r))
                    ctmp = sC.enter_context(tc.tile_pool(name="ctmp", bufs=2))
                    cps = sC.enter_context(
                        tc.tile_pool(name="pc_ps", bufs=2, space="PSUM"))
                    for tt in range(8):
                        xo = ctmp.tile([128, D], f32, tag="xo", name="xo")
                        nc.sync.dma_start(
                            xo[:], din["xp"][tt * 128:(tt + 1) * 128, :])
                        x1 = cpersist.tile([128, D], f32, tag=f"x1_{tt}",
                                           name=f"x1_{tt}")
                        for nb in range(2):
                            yp = cps.tile([128, 384], f32, tag="yp", name="yp")
                            for dc in range(DC):
                                nc.tensor.matmul(
                                    yp[:],
                                    attT[:, dc, tt * 128:(tt + 1) * 128],
                                    wo_sb[:, dc, nb * 384:(nb + 1) * 384],
                                    start=(dc == 0), stop=(dc == DC - 1))
                            nc.vector.tensor_add(
                                x1[:, nb * 384:(nb + 1) * 384], yp[:],
                                xo[:, nb * 384:(nb + 1) * 384])
                        nc.sync.dma_start(x1dbg[tt * 128:(tt + 1) * 128, :],
                                          x1[:])
                        xn2raw = _ln(nc, ctmp, x1, eps_col, xo)
                        xn2 = cpersist.tile([128, D], f32, tag=f"xn2_{tt}",
                                            name=f"xn2_{tt}")
                        nc.vector.tensor_copy(xn2[:], xn2raw[:])
                        tp4 = cps.tile([128, 512], f32, tag="tp2", name="tp2")
                        for i in range(4):
                            nc.tensor.transpose(
                                tp4[:, i * 128:(i + 1) * 128],
                                xn2[:, i * 128:(i + 1) * 128], ident[:])
                        nc.vector.tensor_copy(
                            xn2T[:, 0:4, tt * 128:(tt + 1) * 128],
                            tp4[:].rearrange("p (a n) -> p a n", a=4))
                        tp2 = cps.tile([128, 256], f32, tag="tp2c",
                                       name="tp2c")
                        for i in range(2):
                            nc.tensor.transpose(
                                tp2[:, i * 128:(i + 1) * 128],
                                xn2[:, (4 + i) * 128:(5 + i) * 128], ident[:])
                        nc.vector.tensor_copy(
                            xn2T[:, 4:6, tt * 128:(tt + 1) * 128],
                            tp2[:].rearrange("p (a n) -> p a n", a=2))
                        x1_t.append(x1)
                        xn2_t.append(xn2)

            # ---------- Phase D: router ----------
            rpool = top.enter_context(tc.tile_pool(name="rpool", bufs=1,
                                                   side="right"))
            rps = sCF.enter_context(tc.tile_pool(name="pd_ps", bufs=1,
                                                 space="PSUM"))
            cnt_ps = rps.tile([9, 8], f32)
            with ExitStack() as pd:
                dps = pd.enter_context(
                    tc.tile_pool(name="pd_ps2", bufs=2, space="PSUM"))
                dpool = pd.enter_context(tc.tile_pool(name="pd_tmp", bufs=2))
                dkeep = pd.enter_context(tc.tile_pool(name="pd_keep", bufs=1))
                rt_t, zp1_t, sp0_t, nt_t = [], [], [], []
                # pass 1: noise DMAs + router logits + Exp (one table)
                for tt in range(8):
                    nt = dkeep.tile([128, 8], f32, tag=f"nt_{tt}",
                                    name=f"nt_{tt}")
                    nc.sync.dma_start(
                        nt[:], din["noise"][tt * 128:(tt + 1) * 128, :])
                    nt_t.append(nt)
                for tt in range(8):
                    rp = dps.tile([128, 17], f32, tag="rp", name="rp")
                    for dc in range(DC):
                        nc.tensor.matmul(
                            rp[:], xn2T[:, dc, tt * 128:(tt + 1) * 128],
                            wrout_sb[:, dc, :],
                            start=(dc == 0), stop=(dc == DC - 1))
                    rt = dkeep.tile([128, 17], f32, tag=f"rt_{tt}",
                                    name=f"rt_{tt}")
                    nc.vector.tensor_add(rt[:], rp[:], rbias_bc[:])
                    z = dpool.tile([128, 8], f32, tag="z", name="z")
                    nc.scalar.activation(z[:], rt[:, 8:16], AF.Exp)
                    zp1 = dkeep.tile([128, 8], f32, tag=f"zp1_{tt}",
                                     name=f"zp1_{tt}")
                    nc.vector.tensor_scalar_add(zp1[:], z[:], 1.0)
                    rt_t.append(rt); zp1_t.append(zp1)
                # pass 2: Ln (one table)
                for tt in range(8):
                    sp0 = dkeep.tile([128, 8], f32, tag=f"sp0_{tt}",
                                     name=f"sp0_{tt}")
                    nc.scalar.activation(sp0[:], zp1_t[tt][:], AF.Ln)
                    sp0_t.append(sp0)
                # pass 3: Exp (one table) + vector chain
                for tt in range(8):
                    rt, zp1, sp0, nt = (rt_t[tt], zp1_t[tt], sp0_t[tt],
                                        nt_t[tt])
                    en = dpool.tile([128, 8], f32, tag="en", name="en")
                    nc.scalar.activation(en[:], sp0[:], AF.Exp, scale=-1.0)
                    t1 = dpool.tile([128, 8], f32, tag="t1", name="t1")
                    nc.vector.tensor_tensor(out=t1[:], in0=zp1[:], in1=en[:],
                                            op=ALU.mult)
                    nc.vector.tensor_scalar_add(t1[:], t1[:], -1.0)
                    sp = dpool.tile([128, 8], f32, tag="sp", name="sp")
                    nc.vector.tensor_add(sp[:], sp0[:], t1[:])
                    nm = dpool.tile([128, 8], f32, tag="nm", name="nm")
                    nc.vector.tensor_tensor(out=nm[:], in0=nt[:], in1=sp[:],
                                            op=ALU.mult)
                    noisy = dpool.tile([128, 8], f32, tag="noisy", name="noisy")
                    nc.vector.tensor_add(noisy[:], rt[:, 0:8], nm[:])
                    t8 = dpool.tile([128, 8], f32, tag="t8", name="t8")
                    nc.vector.max(t8[:], noisy[:])
                    ix = dpool.tile([128, 8], u32, tag="ix", name="ix")
                    nc.vector.max_index(ix[:], t8[:], noisy[:])
                    ixf = dpool.tile([128, 8], f32, tag="ixf", name="ixf")
                    nc.vector.tensor_copy(ixf[:], ix[:])
                    dv = dpool.tile([128, 1], f32, tag="dv", name="dv")
                    nc.vector.tensor_sub(dv[:], t8[:, 1:2], t8[:, 0:1])
                    ge = dpool.tile([128, 1], f32, tag="ge", name="ge")
                    nc.scalar.activation(ge[:], dv[:], AF.Exp)
                    gp1 = dpool.tile([128, 1], f32, tag="gp1", name="gp1")
                    nc.vector.tensor_scalar_add(gp1[:], ge[:], 1.0)
                    g1 = rpool.tile([128, 1], f32, tag=f"g1_{tt}",
                                    name=f"g1_{tt}")
                    nc.vector.reciprocal(g1[:], gp1[:])
                    g2 = rpool.tile([128, 1], f32, tag=f"g2_{tt}",
                                    name=f"g2_{tt}")
                    nc.vector.tensor_tensor(out=g2[:], in0=ge[:], in1=g1[:],
                                            op=ALU.mult)
                    ns = rpool.tile([128, 1], f32, tag=f"ns_{tt}",
                                    name=f"ns_{tt}")
                    nc.vector.tensor_scalar(ns[:], rt[:, 16:17], 0.0, None,
                                            ALU.is_le)
                    m0 = rpool.tile([128, 8], f32, tag=f"m0_{tt}",
                                    name=f"m0_{tt}")
                    nc.vector.tensor_scalar(m0[:], iota8[:], ixf[:, 0:1], None,
                                            ALU.is_equal)
                    m1 = rpool.tile([128, 8], f32, tag=f"m1_{tt}",
                                    name=f"m1_{tt}")
                    nc.vector.tensor_scalar(m1[:], iota8[:], ixf[:, 1:2], None,
                                            ALU.is_equal)
                    gf = rpool.tile([128, 8], f32, tag=f"gf_{tt}",
                                    name=f"gf_{tt}")
                    ga = dpool.tile([128, 8], f32, tag="ga", name="ga")
                    nc.vector.tensor_scalar(ga[:], m0[:], g1[:, :1], None,
                                            ALU.mult)
                    gb = dpool.tile([128, 8], f32, tag="gb", name="gb")
                    nc.vector.tensor_scalar(gb[:], m1[:], g2[:, :1], None,
                                            ALU.mult)
                    nc.vector.tensor_add(gf[:], ga[:], gb[:])
                    m = rpool.tile([128, 8], f32, tag=f"m_{tt}", name=f"m_{tt}")
                    nc.vector.tensor_add(m[:], m0[:], m1[:])
                    nc.vector.tensor_scalar_min(m[:], m[:], 1.0)
                    nc.vector.tensor_scalar(m[:], m[:], ns[:, :1], None,
                                            ALU.mult)
                    maug = rpool.tile([128, 9], f32, tag=f"maug_{tt}",
                                      name=f"maug_{tt}")
                    nc.vector.tensor_copy(maug[:, 0:8], m[:])
                    nc.vector.tensor_copy(maug[:, 8:9], ns[:])
                    nc.tensor.matmul(cnt_ps[:, tt:tt + 1], maug[:], ones_r[:],
                                     start=True, stop=True)
                    g1p = rpool.tile([128, 1], f32, tag=f"g1p_{tt}",
                                     name=f"g1p_{tt}")
                    nc.vector.tensor_tensor(out=g1p[:], in0=g1[:], in1=ns[:],
                                            op=ALU.mult)
                    g2p = rpool.tile([128, 1], f32, tag=f"g2p_{tt}",
                                     name=f"g2p_{tt}")
                    nc.vector.tensor_tensor(out=g2p[:], in0=g2[:], in1=ns[:],
                                            op=ALU.mult)
                    nsp = rpool.tile([128, 1], f32, tag=f"nsp_{tt}",
                                     name=f"nsp_{tt}")
                    nc.vector.tensor_scalar(nsp[:], ns[:], 1.0, -1.0,
                                            ALU.subtract, ALU.mult)
                    g1_t.append(g1p); g2_t.append(g2p); ns_t.append(nsp)
                    m0_t.append(m0); m1_t.append(m1); m_t.append(m)
                    maug_t.append(maug); gf_t.append(gf)

            cnt_sb = rpool.tile([9, 8], f32)
            nc.vector.tensor_copy(cnt_sb[:], cnt_ps[:])
            nc.sync.dma_start(cc_in[:], cnt_sb[:])
            nc.gpsimd.collective_compute(
                "AllGather", ALU.bypass, replica_groups=[list(range(NCORES))],
                ins=[cc_in.opt()], outs=[cc_out.opt()])
            cnts_all = rpool.tile([72, 8], f32)
            nc.sync.dma_start(cnts_all[:], cc_out[:])

            # ---------- Phase E ----------
            flat = rpool.tile([9, 64], f32)
            for r in range(NCORES):
                b2, a = r // 2, r % 2
                if a == 0:
                    nc.sync.dma_start(flat[:, b2 * 16:b2 * 16 + 4],
                                      cnts_all[9 * r:9 * r + 9, 0:4])
                    nc.sync.dma_start(flat[:, b2 * 16 + 12:b2 * 16 + 16],
                                      cnts_all[9 * r:9 * r + 9, 4:8])
                else:
                    nc.sync.dma_start(flat[:, b2 * 16 + 4:b2 * 16 + 12],
                                      cnts_all[9 * r:9 * r + 9, 0:8])
            zf = rpool.tile([9, 64], f32)
            nc.vector.memset(zf[:], 0.0)
            incl = rpool.tile([9, 64], f32)
            nc.vector.tensor_tensor_scan(incl[:], flat[:], zf[:], 0.0,
                                         ALU.add, ALU.add)
            excl = rpool.tile([9, 64], f32)
            nc.vector.tensor_sub(excl[:], incl[:], flat[:])
            tot = rpool.tile([1, 1], f32)
            nc.sync.dma_start(tot[:], incl[8:9, 63:64])
            tot_i = rpool.tile([1, 1], i32)
            nc.vector.tensor_copy(tot_i[:], tot[:])
            cap_i = rpool.tile([1, 1], i32)
            nc.vector.tensor_scalar(cap_i[:], tot_i[:], 2, None,
                                    ALU.arith_shift_right)
            capt = rpool.tile([1, 1], f32)
            nc.vector.tensor_copy(capt[:], cap_i[:])
            cap_bc = rpool.tile([128, 1], f32)
            nc.gpsimd.partition_broadcast(cap_bc[:], capt[:])

            exT_ps = rps.tile([64, 9], f32)
            nc.tensor.transpose(exT_ps[:], excl[:, 0:64], ident[0:9, 0:9])
            exT = rpool.tile([64, 9], f32)
            nc.vector.tensor_copy(exT[:], exT_ps[:])
            myo_ps = rps.tile([9, 8], f32)
            nc.tensor.matmul(myo_ps[:], exT[:, 0:9], chsel_sb[:], start=True,
                             stop=True)
            myo = rpool.tile([9, 8], f32)
            nc.vector.tensor_copy(myo[:], myo_ps[:])
            myoT_ps = rps.tile([8, 9], f32)
            nc.tensor.transpose(myoT_ps[:], myo[:], ident[0:9, 0:9])
            myoT = rpool.tile([8, 9], f32)
            nc.vector.tensor_copy(myoT[:], myoT_ps[:])

            # ---------- Phase F ----------
            kcnt_ps = rps.tile([8, 8], f32)
            with ExitStack() as pf:
                fps = pf.enter_context(
                    tc.tile_pool(name="pf_ps", bufs=2, space="PSUM"))
                for tt in range(8):
                    pr = fps.tile([128, 8], f32, tag="pr", name="pr")
                    orow = rpool.tile([1, 8], f32, tag=f"orow_{tt}",
                                      name=f"orow_{tt}")
                    nc.sync.dma_start(orow[:], myoT[tt:tt + 1, 0:8])
                    nc.tensor.matmul(pr[:], ones1r[:], orow[:],
                                     start=True, stop=False)
                    nc.tensor.matmul(pr[:], lincl[:], maug_t[tt][:, 0:8],
                                     start=False, stop=True)
                    keepb = rpool.tile([128, 8], f32, tag=f"kb_{tt}",
                                       name=f"kb_{tt}")
                    nc.vector.tensor_scalar(keepb[:], pr[:], cap_bc[:, :1],
                                            None, ALU.is_le)
                    keep = rpool.tile([128, 8], f32, tag=f"keep_{tt}",
                                      name=f"keep_{tt}")
                    nc.vector.tensor_tensor(out=keep[:], in0=keepb[:],
                                            in1=m_t[tt][:], op=ALU.mult)
                    nc.tensor.matmul(kcnt_ps[:, tt:tt + 1], keep[:],
                                     ones_r[:], start=True, stop=True)
                    kg = rpool.tile([128, 8], f32, tag=f"kg_{tt}",
                                    name=f"kg_{tt}")
                    nc.vector.tensor_tensor(out=kg[:], in0=keep[:],
                                            in1=gf_t[tt][:], op=ALU.mult)
                    nc.sync.dma_start(rdbg[tt * 128:(tt + 1) * 128, :], kg[:])
                    keep_t.append(keep); keepr_t.append(keep)

            kcnt = rpool.tile([8, 8], f32)
            nc.vector.tensor_copy(kcnt[:], kcnt_ps[:])
            zf8 = rpool.tile([8, 8], f32)
            nc.vector.memset(zf8[:], 0.0)
            kincl = rpool.tile([8, 8], f32)
            nc.vector.tensor_tensor_scan(kincl[:], kcnt[:], zf8[:], 0.0,
                                         ALU.add, ALU.add)
            kexcl = rpool.tile([8, 8], f32)
            nc.vector.tensor_sub(kexcl[:], kincl[:], kcnt[:])
            kexT_ps = rps.tile([8, 8], f32)
            nc.tensor.transpose(kexT_ps[:], kexcl[:], ident[0:8, 0:8])
            kexT = rpool.tile([8, 8], f32)
            nc.vector.tensor_copy(kexT[:], kexT_ps[:])

            with ExitStack() as pf2:
                f2ps = pf2.enter_context(
                    tc.tile_pool(name="pf2_ps", bufs=2, space="PSUM"))
                f2p = pf2.enter_context(tc.tile_pool(name="pf2_sb", bufs=2))
                for tt in range(8):
                    p2 = f2ps.tile([128, 8], f32, tag="p2", name="p2")
                    krow = rpool.tile([1, 8], f32, tag=f"krow_{tt}",
                                      name=f"krow_{tt}")
                    nc.sync.dma_start(krow[:], kexT[tt:tt + 1, :])
                    nc.tensor.matmul(p2[:], ones1r[:], krow[:],
                                     start=True, stop=False)
                    nc.tensor.matmul(p2[:], lstrict[:], keepr_t[tt][:],
                                     start=False, stop=True)
                    slotf = f2p.tile([128, 8], f32, tag="slotf", name="slotf")
                    nc.vector.tensor_add(slotf[:], p2[:], iotase[:])
                    gidx = rpool.tile([128, 2], i32, tag=f"gi_{tt}",
                                      name=f"gi_{tt}")
                    for k, mk in ((0, m0_t[tt]), (1, m1_t[tt])):
                        fim = f2p.tile([128, 8], f32, tag="fim", name="fim")
                        nc.vector.tensor_tensor(out=fim[:], in0=slotf[:],
                                                in1=mk[:], op=ALU.mult)
                        fi = f2p.tile([128, 1], f32, tag="fi", name="fi")
                        nc.vector.reduce_sum(fi[:], fim[:],
                                             axis=mybir.AxisListType.X)
                        km = f2p.tile([128, 8], f32, tag="km", name="km")
                        nc.vector.tensor_tensor(out=km[:], in0=mk[:],
                                                in1=keep_t[tt][:], op=ALU.mult)
                        kept = f2p.tile([128, 1], f32, tag="kept", name="kept")
                        nc.vector.reduce_sum(kept[:], km[:],
                                             axis=mybir.AxisListType.X)
                        u = f2p.tile([128, 1], f32, tag="u", name="u")
                        nc.vector.tensor_scalar_add(u[:], kept[:], -1.0)
                        nc.vector.tensor_scalar_mul(u[:], u[:], -70000.0)
                        fis = f2p.tile([128, 1], f32, tag="fis", name="fis")
                        nc.vector.tensor_add(fis[:], fi[:], u[:])
                        fii = f2p.tile([128, 1], i32, tag="fii", name="fii")
                        nc.vector.tensor_copy(fii[:], fis[:])
                        nc.gpsimd.indirect_dma_start(
                            out=xe_dram[:],
                            out_offset=bass.IndirectOffsetOnAxis(
                                ap=fii[:, :1], axis=0),
                            in_=xn2_t[tt][:], in_offset=None,
                            bounds_check=E * SE - 1, oob_is_err=False)
                        gi = f2p.tile([128, 1], f32, tag="gi2", name="gi2")
                        nc.vector.tensor_tensor(out=gi[:], in0=fi[:],
                                                in1=kept[:], op=ALU.mult)
                        w = f2p.tile([128, 1], f32, tag="u2", name="u2")
                        nc.vector.tensor_scalar_add(w[:], kept[:], -1.0)
                        nc.vector.tensor_scalar_mul(w[:], w[:], -float(ZROW))
                        nc.vector.tensor_add(gi[:], gi[:], w[:])
                        nc.vector.tensor_copy(gidx[:, k:k + 1], gi[:])
                    gidx_t.append(gidx)

        # ---------- Phase G: expert MLPs ----------
        zrow = rpool.tile([128, D], f32)
        nc.vector.memset(zrow[:], 0.0)
        nc.sync.dma_start(ye_dram[ZROW:ZROW + 1, :], zrow[0:1, :])
        with ExitStack() as pg:
            gsb = pg.enter_context(tc.tile_pool(name="pg_sb", bufs=2))
            xetp = pg.enter_context(tc.tile_pool(name="pg_xet", bufs=1))
            w1p = pg.enter_context(tc.tile_pool(name="pg_w1", bufs=12))
            w2p = pg.enter_context(tc.tile_pool(name="pg_w2", bufs=12))
            hpool = pg.enter_context(tc.tile_pool(name="pg_h", bufs=2))
            ROWS = [(0, 128), (128, 64)]
            xet = xetp.tile([128, E, DC, SE], f8)
            with ExitStack() as pgt:
                tps = pgt.enter_context(
                    tc.tile_pool(name="pg_tps", bufs=2, space="PSUM"))
                for e in range(E):
                    for r0, rn in ROWS:
                        xe = gsb.tile([128, D], f32, tag="xe", name="xe")
                        nc.sync.dma_start(
                            xe[0:rn, :],
                            xe_dram[e * SE + r0:e * SE + r0 + rn, :])
                        tp4 = tps.tile([128, 512], f32, tag="tp3", name="tp3")
                        for i in range(4):
                            nc.tensor.transpose(
                                tp4[:, i * 128:i * 128 + rn],
                                xe[0:rn, i * 128:(i + 1) * 128],
                                ident[0:rn, 0:rn])
                        nc.vector.tensor_scalar_mul(
                            xet[:, e, 0:4, r0:r0 + rn],
                            tp4[:].rearrange("p (a n) -> p a n",
                                             a=4)[:, :, 0:rn],
                            XSCALE)
                        tp2 = tps.tile([128, 256], f32, tag="tp3b",
                                       name="tp3b")
                        for i in range(2):
                            nc.tensor.transpose(
                                tp2[:, i * 128:i * 128 + rn],
                                xe[0:rn, (4 + i) * 128:(5 + i) * 128],
                                ident[0:rn, 0:rn])
                        nc.vector.tensor_scalar_mul(
                            xet[:, e, 4:6, r0:r0 + rn],
                            tp2[:].rearrange("p (a n) -> p a n",
                                             a=2)[:, :, 0:rn],
                            XSCALE)
            gps = pg.enter_context(
                tc.tile_pool(name="pg_ps", bufs=2, space="PSUM"))
            yps = pg.enter_context(
                tc.tile_pool(name="pg_yps", bufs=1, space="PSUM"))
            for e in range(E):
                hT = hpool.tile([128, 24, SE], f8, tag="hT", name="hT")
                for q in range(6):
                    if e == 0:
                        w1t = w1e0[q]
                    else:
                        w1t = w1p.tile([128, 4, DC, 128], f8, tag="w1t",
                                       name="w1t")
                        nc.sync.dma_start(
                            w1t[:],
                            din["we1"][e, q].rearrange(
                                "p (i a n) -> p i a n", i=4, a=DC))
                    for i in range(4):
                        mt = 4 * q + i
                        hp = gps.tile([128, SE], f32, tag="hp", name="hp")
                        for dh in range(3):
                            nc.tensor.matmul(
                                hp[:], w1t[:, i, 2 * dh:2 * dh + 2, :],
                                xet[:, e, 2 * dh:2 * dh + 2, :],
                                start=(dh == 0), stop=(dh == 2),
                                perf_mode=DR)
                        nc.vector.tensor_scalar(hT[:, mt, :], hp[:], 0.0,
                                                1.0 / WSCALE, ALU.max,
                                                ALU.mult)
                ypl = [yps.tile([128, 384], f32, tag=f"yp_{i}",
                                name=f"ypl_{i}") for i in range(4)]
                for q in range(6):
                    if e == 0:
                        w2t = w2e0[q]
                    else:
                        w2t = w2p.tile([128, 4, D], f8, tag="w2t",
                                       name="w2t")
                        nc.sync.dma_start(
                            w2t[:],
                            din["we2"][e, q].rearrange("p (i n) -> p i n",
                                                       i=4))
                    for ip in range(2):
                        hc0 = 4 * q + 2 * ip
                        for rt2, (r0, rn) in enumerate(ROWS):
                            for nb in range(2):
                                nc.tensor.matmul(
                                    ypl[rt2 * 2 + nb][0:rn, :],
                                    hT[:, hc0:hc0 + 2, r0:r0 + rn],
                                    w2t[:, 2 * ip:2 * ip + 2,
                                        nb * 384:(nb + 1) * 384],
                                    start=(hc0 == 0), stop=(hc0 == 22),
                                    perf_mode=DR)
                for rt2, (r0, rn) in enumerate(ROWS):
                    ysb = gsb.tile([128, D], f32, tag="ysb", name="ysb")
                    for nb in range(2):
                        nc.vector.tensor_scalar_mul(
                            ysb[0:rn, nb * 384:(nb + 1) * 384],
                            ypl[rt2 * 2 + nb][0:rn, :],
                            1.0 / (XSCALE * WSCALE))
                    nc.sync.dma_start(
                        ye_dram[e * SE + r0:e * SE + r0 + rn, :],
                        ysb[0:rn, :])

        # ---------- Phase H ----------
        with ExitStack() as ph:
            hsb = ph.enter_context(tc.tile_pool(name="ph_sb", bufs=3))
            for tt in range(8):
                yg0 = hsb.tile([128, D], f32, tag="yg0", name="yg0")
                nc.gpsimd.indirect_dma_start(
                    out=yg0[:], out_offset=None, in_=ye_dram[:],
                    in_offset=bass.IndirectOffsetOnAxis(
                        ap=gidx_t[tt][:, 0:1], axis=0))
                yg1 = hsb.tile([128, D], f32, tag="yg1", name="yg1")
                nc.gpsimd.indirect_dma_start(
                    out=yg1[:], out_offset=None, in_=ye_dram[:],
                    in_offset=bass.IndirectOffsetOnAxis(
                        ap=gidx_t[tt][:, 1:2], axis=0))
                u0 = hsb.tile([128, D], f32, tag="u0", name="u0")
                nc.vector.tensor_scalar(u0[:], yg0[:], g1_t[tt][:, :1], None,
                                        ALU.mult)
                u1 = hsb.tile([128, D], f32, tag="u1", name="u1")
                nc.scalar.activation(u1[:], yg1[:], AF.Copy,
                                     scale=g2_t[tt][:, :1])
                w = hsb.tile([128, D], f32, tag="w", name="w")
                nc.vector.tensor_scalar(w[:], xn2_t[tt][:], ns_t[tt][:, :1],
                                        None, ALU.mult)
                s01 = hsb.tile([128, D], f32, tag="s01", name="s01")
                nc.vector.tensor_add(s01[:], u0[:], u1[:])
                wx = hsb.tile([128, D], f32, tag="wx", name="wx")
                nc.vector.tensor_add(wx[:], w[:], x1_t[tt][:])
                out = hsb.tile([128, D], f32, tag="out", name="out")
                nc.vector.tensor_add(out[:], s01[:], wx[:])
                nc.sync.dma_start(yout[tt * 128:(tt + 1) * 128, :], out[:])

    nc.compile()
    return nc


_OWN = {0: [0, 1, 2, 3, 12, 13, 14, 15], 1: [4, 5, 6, 7, 8, 9, 10, 11]}


def _core_meta(c):
    b, a = c // 2, c % 2
    own = _OWN[a]
    other = [g for g in range(16) if g not in own]
    perm_chunks = own + other
    rows = np.concatenate([np.arange(g * 128, (g + 1) * 128)
                           for g in perm_chunks])
    return b, a, own, rows


def _host_inputs(x, noise, Wq, Wk, Wv, Wo, Wr, br, Wn, bn, Wsk, bsk, We1, We2,
                 **_unused):
    x = np.asarray(x, np.float32)
    noise = np.asarray(noise, np.float32)
    wq = np.ascontiguousarray(
        (np.transpose(np.asarray(Wq), (1, 0, 2)).reshape(D, D)
         * np.float32(D ** -0.5)).astype(np.float32))
    wk = np.ascontiguousarray(
        np.transpose(np.asarray(Wk), (1, 0, 2)).reshape(D, D)
        .astype(np.float32))
    wv = np.ascontiguousarray(
        np.transpose(np.asarray(Wv), (1, 0, 2)).reshape(D, D)
        .astype(np.float32))
    wrout = np.ascontiguousarray(np.concatenate(
        [np.asarray(Wr), np.asarray(Wn), np.asarray(Wsk)], axis=1)
        .astype(np.float32))
    rbias = np.concatenate(
        [np.asarray(br), np.asarray(bn), np.asarray(bsk)])[None, :] \
        .astype(np.float32)
    qsel = np.zeros((2, NOWN), np.float32)
    qsel[0, 0:512] = 1.0
    qsel[1, 512:1024] = 1.0
    import ml_dtypes
    W1 = (np.asarray(We1, np.float32) * WSCALE).reshape(E, DC, 128, 6, 4, 128)
    we1 = np.ascontiguousarray(
        W1.transpose(0, 3, 2, 4, 1, 5).reshape(E, 6, 128, 3072)
        .astype(ml_dtypes.float8_e4m3))
    W2 = (np.asarray(We2, np.float32) * WSCALE).reshape(E, 6, 4, 128, D)
    we2 = np.ascontiguousarray(
        W2.transpose(0, 1, 3, 2, 4).reshape(E, 6, 128, 3072)
        .astype(ml_dtypes.float8_e4m3))
    wo = np.ascontiguousarray(np.asarray(Wo, np.float32))

    in_maps = []
    for c in range(NCORES):
        b, a, own, rows = _core_meta(c)
        gid = rows
        kbias = np.zeros((2, T), np.float32)
        for s in range(2):
            qmax = gid[s * 512:(s + 1) * 512].max()
            kbias[s] = np.where(gid > qmax, NEG, 0.0).astype(np.float32)
        chunksel = np.zeros((64, E), np.float32)
        for lc in range(8):
            chunksel[b * 16 + own[lc], lc] = 1.0
        in_maps.append({
            "xp": np.ascontiguousarray(x[b][rows]),
            "noise": np.ascontiguousarray(noise[b][rows[:NOWN]]),
            "wq": wq, "wk": wk, "wv": wv, "wo": wo,
            "wrout": wrout, "rbias": rbias,
            "we1": we1, "we2": we2,
            "kbias": kbias, "qsel": qsel,
            "chunksel": chunksel,
        })
    return in_maps


_prog = None


def run(trace=False, **inputs):
    global _prog
    if _prog is None:
        _prog = build_program()
    in_maps = _host_inputs(**inputs)
    res = bass_utils.run_bass_kernel_spmd(
        _prog, in_maps, core_ids=list(range(NCORES)), trace=trace)
    out = np.zeros((B, T, D), np.float32)
    for c in range(NCORES):
        b, a, own, rows = _core_meta(c)
        out[b][rows[:NOWN]] = res.results[c]["yout"]
    return out, res


def kernel(**inputs):
    out, _ = run(trace=False, **inputs)
    return out


# revision 29
# speedup vs baseline: 1.1392x; 1.0344x over previous
"""CrossLayerBlock kernel: baseline + fp8-DoubleRow expert MLPs."""
import numpy as np
from contextlib import ExitStack

import concourse.bass as bass
import concourse.tile as tile
from concourse import bacc, mybir
from concourse import bass_utils
from concourse.masks import make_identity

B, T, D, H, HS, E = 4, 2048, 768, 12, 64, 8
NCORES = 8
NOWN = 1024
NKC = 16
DC = 6
SE = 192
ZROW = E * SE
LN_EPS = 1e-5
NEG = -30.0

f32 = mybir.dt.float32
f32r = mybir.dt.float32r
bf16 = mybir.dt.bfloat16
f8 = mybir.dt.float8e4
WSCALE = 64.0
XSCALE = 16.0
i32 = mybir.dt.int32
u32 = mybir.dt.uint32
AF = mybir.ActivationFunctionType
ALU = mybir.AluOpType
DR = mybir.MatmulPerfMode.DoubleRow

KCS0 = [8, 9, 10, 11, 0, 1, 2, 3]
KCS1 = [8, 9, 10, 11, 12, 13, 14, 15, 0, 1, 2, 3, 4, 5, 6, 7]


def _ln(nc, pool, xt, eps_col, scratch):
    r = pool.tile([128, 1], f32, tag="ln_r", name="ln_r")
    nc.vector.reduce_sum(r[:], xt[:], axis=mybir.AxisListType.X)
    sq = scratch
    r2 = pool.tile([128, 1], f32, tag="ln_r2", name="ln_r2")
    nc.gpsimd.tensor_tensor(out=sq[:], in0=xt[:], in1=xt[:], op=ALU.mult)
    nc.vector.reduce_sum(r2[:], sq[:], axis=mybir.AxisListType.X)
    mu = pool.tile([128, 1], f32, tag="ln_mu", name="ln_mu")
    nc.vector.tensor_scalar_mul(mu[:], r[:], 1.0 / D)
    mr = pool.tile([128, 1], f32, tag="ln_mr", name="ln_mr")
    nc.vector.tensor_tensor(out=mr[:], in0=mu[:], in1=r[:], op=ALU.mult)
    vd = pool.tile([128, 1], f32, tag="ln_vd", name="ln_vd")
    nc.vector.tensor_sub(vd[:], r2[:], mr[:])
    sd = pool.tile([128, 1], f32, tag="ln_sd", name="ln_sd")
    nc.scalar.activation(sd[:], vd[:], AF.Sqrt, bias=eps_col[:, :1],
                         scale=1.0 / D)
    rstd = pool.tile([128, 1], f32, tag="ln_rstd", name="ln_rstd")
    nc.vector.reciprocal(rstd[:], sd[:])
    xn = pool.tile([128, D], f32, tag="ln_xn", name="ln_xn")
    nc.vector.tensor_scalar(xn[:], xt[:], mu[:, :1], rstd[:, :1],
                            ALU.subtract, ALU.mult)
    return xn


def build_program():
    nc = bacc.Bacc("TRN2", target_bir_lowering=False, debug=False,
                   enable_asserts=False, num_devices=NCORES)

    din = {}
    for name, shape, dt in [
        ("xp", [T, D], f32), ("noise", [NOWN, E], f32),
        ("wqh", [H, 128, DC * 64], f32), ("wkh", [H, 128, DC * 64], f32),
        ("wv", [D, D], f32),
        ("wo", [D, D], f32), ("wrout", [D, 17], f32), ("rbias", [1, 17], f32),
        ("we1", [E, 6, 128, 3072], f8), ("we2", [E, 6, 128, 3072], f8),
        ("kbias", [2, T], f32), ("qsel", [2, NOWN], f32),
        ("chunksel", [64, E], f32),
    ]:
        din[name] = nc.dram_tensor(name, shape, dt, kind="ExternalInput").ap()

    yout = nc.dram_tensor("yout", [NOWN, D], f32, kind="ExternalOutput").ap()
    x1dbg = nc.dram_tensor("x1dbg", [NOWN, D], f32, kind="ExternalOutput").ap()
    rdbg = nc.dram_tensor("rdbg", [NOWN, E], f32, kind="ExternalOutput").ap()

    with tile.TileContext(nc) as tc, ExitStack() as top:
        dram = top.enter_context(tc.tile_pool(name="dram", bufs=1, space="DRAM"))
        xe_dram = dram.tile([E * SE, D], f32)
        ye_dram = dram.tile([E * SE + 1, D], f32)
        cc_in = dram.tile([9, 8], f32)
        cc_out = dram.tile([72, 8], f32, addr_space="Shared")

        const = top.enter_context(tc.tile_pool(name="const", bufs=1))
        ident = const.tile([128, 128], f32)
        make_identity(nc, ident[:])
        lincl = const.tile([128, 128], f32)
        nc.gpsimd.memset(lincl[:], 1.0)
        nc.gpsimd.affine_select(out=lincl[:], in_=lincl[:],
                                compare_op=ALU.is_ge, fill=0.0, base=0,
                                pattern=[[1, 128]], channel_multiplier=-1)
        lstrict = const.tile([128, 128], f32)
        nc.gpsimd.memset(lstrict[:], 1.0)
        nc.gpsimd.affine_select(out=lstrict[:], in_=lstrict[:],
                                compare_op=ALU.is_gt, fill=0.0, base=0,
                                pattern=[[1, 128]], channel_multiplier=-1)
        dm = []
        for d in range(4):
            dmf = const.tile([128, 512], f32, tag="dmf", name=f"dmf_{d}")
            nc.gpsimd.memset(dmf[:], 1.0)
            nc.gpsimd.affine_select(out=dmf[:], in_=dmf[:],
                                    compare_op=ALU.is_ge, fill=0.0,
                                    base=-d * 128, pattern=[[1, 512]],
                                    channel_multiplier=-1)
            dmb = const.tile([128, 512], bf16, tag=f"dmb_{d}", name=f"dmb_{d}")
            nc.vector.tensor_copy(dmb[:], dmf[:])
            dm.append(dmb)
        ones_r = const.tile([128, 1], f32)
        nc.vector.memset(ones_r[:], 1.0)
        ones1r = const.tile([1, 128], f32)
        nc.vector.memset(ones1r[:], 1.0)
        iota8i = const.tile([128, 8], i32)
        nc.gpsimd.iota(iota8i[:], pattern=[[1, 8]], base=0, channel_multiplier=0)
        iota8 = const.tile([128, 8], f32)
        nc.vector.tensor_copy(iota8[:], iota8i[:])
        iotase = const.tile([128, 8], f32)
        nc.vector.tensor_scalar_mul(iotase[:], iota8[:], float(SE))
        eps_col = const.tile([128, 1], f32)
        nc.vector.memset(eps_col[:], LN_EPS)
        rbias_bc = const.tile([128, 17], f32)
        rb1 = const.tile([1, 17], f32)
        nc.sync.dma_start(rb1[:], din["rbias"][:])
        nc.gpsimd.partition_broadcast(rbias_bc[:], rb1[:])
        wrout_sb = const.tile([128, DC, 17], f32)
        nc.sync.dma_start(wrout_sb[:],
                          din["wrout"].rearrange("(a p) n -> p a n", p=128))
        chsel_sb = const.tile([64, E], f32)
        nc.sync.dma_start(chsel_sb[:], din["chunksel"][:])

        x1_t, xn2_t = [], []
        g1_t, g2_t, ns_t, m0_t, m1_t, m_t, maug_t, gf_t = ([] for _ in range(8))
        keep_t, keepr_t, gidx_t = [], [], []

        with ExitStack() as sCF:
            if True:
                pbcp = sCF.enter_context(tc.tile_pool(name="pbcp", bufs=1))
                attT = pbcp.tile([128, DC, NOWN], f32r)
                xn2T = pbcp.tile([128, DC, NOWN], f32)
                with ExitStack() as sAB:
                    pab = sAB.enter_context(tc.tile_pool(name="pab", bufs=1))
                    xnT = pab.tile([128, DC, T], f32r)
                    vaug = pab.tile([128, NKC, H, 128], bf16)

                    # ---------- Phase A ----------
                    with ExitStack() as sA:
                        wvp = sA.enter_context(tc.tile_pool(name="wvp", bufs=1))
                        wv_sb = wvp.tile([128, DC, D], f32r)
                        nc.sync.dma_start(
                            wv_sb[:],
                            din["wv"].rearrange("(a p) n -> p a n",
                                                p=128).bitcast(f32r))
                        apool = sA.enter_context(tc.tile_pool(name="pa_sb",
                                                              bufs=2))
                        aps = sA.enter_context(
                            tc.tile_pool(name="pa_ps", bufs=2, space="PSUM"))
                        for kc in range(NKC):
                            xt = apool.tile([128, D], f32, tag="xt", name="xt")
                            nc.sync.dma_start(
                                xt[:], din["xp"][kc * 128:(kc + 1) * 128, :])
                            lsc = apool.tile([128, D], f32, tag="lsc",
                                             name="lsc")
                            xn = _ln(nc, apool, xt, eps_col, lsc)
                            tp4 = aps.tile([128, 512], f32, tag="tp",
                                           name="tp")
                            for i in range(4):
                                nc.tensor.transpose(
                                    tp4[:, i * 128:(i + 1) * 128],
                                    xn[:, i * 128:(i + 1) * 128], ident[:])
                            nc.vector.tensor_copy(
                                xnT[:, 0:4, kc * 128:(kc + 1) * 128],
                                tp4[:].rearrange("p (a n) -> p a n", a=4))
                            tp2 = aps.tile([128, 256], f32, tag="tp2b",
                                           name="tp2b")
                            for i in range(2):
                                nc.tensor.transpose(
                                    tp2[:, i * 128:(i + 1) * 128],
                                    xn[:, (4 + i) * 128:(5 + i) * 128],
                                    ident[:])
                            nc.vector.tensor_copy(
                                xnT[:, 4:6, kc * 128:(kc + 1) * 128],
                                tp2[:].rearrange("p (a n) -> p a n", a=2))
                            for nb in range(2):
                                vp = aps.tile([128, 384], f32, tag="vp",
                                              name="vp")
                                for dc in range(DC):
                                    nc.tensor.matmul(
                                        vp[:],
                                        xnT[:, dc, kc * 128:(kc + 1) * 128],
                                        wv_sb[:, dc, nb * 384:(nb + 1) * 384],
                                        start=(dc == 0), stop=(dc == DC - 1))
                                nc.vector.tensor_copy(
                                    vaug[:, kc, nb * 6:(nb + 1) * 6, 0:HS],
                                    vp[:].rearrange("p (h e) -> p h e", e=HS))
                        nc.gpsimd.memset(vaug[:, :, :, HS:128], 1.0)

                    # ---------- Phase B ----------
                    with ExitStack() as sB:
                        wqkp = sB.enter_context(tc.tile_pool(name="wqkp",
                                                             bufs=3))
                        kqpool = sB.enter_context(tc.tile_pool(name="pb_kq",
                                                               bufs=2))
                        bpool = sB.enter_context(tc.tile_pool(name="pb_sb",
                                                              bufs=4))
                        recp = sB.enter_context(tc.tile_pool(name="pb_rec",
                                                             bufs=1))
                        bps = sB.enter_context(
                            tc.tile_pool(name="pb_ps", bufs=3, space="PSUM"))
                        atps = sB.enter_context(
                            tc.tile_pool(name="pb_at", bufs=2, space="PSUM"))
                        for h in range(H):
                            wk_t = wqkp.tile([128, DC, 64], f32r, tag="wk_t",
                                             name="wk_t")
                            nc.sync.dma_start(
                                wk_t[:],
                                din["wkh"][h].rearrange(
                                    "p (a n) -> p a n", a=DC).bitcast(f32r))
                            wq_t = wqkp.tile([128, DC, 64], f32r, tag="wq_t",
                                             name="wq_t")
                            nc.sync.dma_start(
                                wq_t[:],
                                din["wqh"][h].rearrange(
                                    "p (a n) -> p a n", a=DC).bitcast(f32r))
                            kT = kqpool.tile([66, T], f32r, tag="kT", name="kT")
                            nc.sync.dma_start(kT[64:66, :],
                                              din["kbias"][:].bitcast(f32r))
                            for qb in range(4):
                                kp = bps.tile([64, 512], f32, tag="kp",
                                              name="kp")
                                for dc in range(DC):
                                    nc.tensor.matmul(
                                        kp[:],
                                        wk_t[:, dc, :],
                                        xnT[:, dc, qb * 512:(qb + 1) * 512],
                                        start=(dc == 0), stop=(dc == DC - 1))
                                nc.vector.tensor_copy(
                                    kT[0:64, qb * 512:(qb + 1) * 512], kp[:])
                            qT = kqpool.tile([66, NOWN], f32r, tag="qT",
                                             name="qT")
                            nc.sync.dma_start(qT[64:66, :],
                                              din["qsel"][:].bitcast(f32r))
                            for qb in range(2):
                                qp = bps.tile([64, 512], f32, tag="kp",
                                              name="qp")
                                for dc in range(DC):
                                    nc.tensor.matmul(
                                        qp[:],
                                        wq_t[:, dc, :],
                                        xnT[:, dc, qb * 512:(qb + 1) * 512],
                                        start=(dc == 0), stop=(dc == DC - 1))
                                nc.vector.tensor_copy(
                                    qT[0:64, qb * 512:(qb + 1) * 512], qp[:])

                            for slot, kcs in ((0, KCS0), (1, KCS1)):
                                at = atps.tile([128, 512], f32, tag="at",
                                               name="at")
                                pend = []
                                for j, kc in enumerate(kcs):
                                    st = bps.tile([128, 512], f32, tag="st",
                                                  name="st")
                                    nc.tensor.matmul(
                                        st[:], kT[:, kc * 128:(kc + 1) * 128],
                                        qT[:, slot * 512:(slot + 1) * 512],
                                        start=True, stop=True)
                                    exr = bpool.tile([128, 512], bf16,
                                                     tag="exr", name="exr")
                                    nc.scalar.activation(exr[:], st[:], AF.Exp)
                                    if kc < 8 and kc // 4 == slot:
                                        nc.vector.tensor_tensor(
                                            out=exr[:], in0=exr[:],
                                            in1=dm[kc % 4][:], op=ALU.mult)
                                    pend.append((kc, exr))
                                    if len(pend) > 1:
                                        kcp, exrp = pend.pop(0)
                                        nc.tensor.matmul(
                                            at[:], vaug[:, kcp, h, :],
                                            exrp[:], start=(j == 1),
                                            stop=False)
                                kcp, exrp = pend.pop(0)
                                nc.tensor.matmul(
                                    at[:], vaug[:, kcp, h, :], exrp[:],
                                    start=False, stop=True)
                                rec = recp.tile([64, 512], f32, tag="rec",
                                                 name="rec")
                                nc.vector.reciprocal(rec[:], at[64:128, :])
                                nc.vector.tensor_tensor(
                                    out=attT[(h % 2) * 64:(h % 2) * 64 + 64,
                                             h // 2,
                                             slot * 512:(slot + 1) * 512],
                                    in0=at[0:64, :], in1=rec[:], op=ALU.mult)

                # ---------- Phase C ----------
                cpersist = top.enter_context(tc.tile_pool(name="cpersist",
                                                          bufs=1,
                                                          side="right"))
                wpre = top.enter_context(tc.tile_pool(name="wpre", bufs=1,
                                                      side="right"))
                with ExitStack() as sC:
                    wop = sC.enter_context(tc.tile_pool(name="wop", bufs=1))
                    wo_sb = wop.tile([128, DC, D], f32r)
                    nc.sync.dma_start(
                        wo_sb[:],
                        din["wo"].rearrange("(a p) n -> p a n",
                                            p=128).bitcast(f32r))
                    w1e0, w2e0 = [], []
                    for q in range(6):
                        t1 = wpre.tile([128, 4, DC, 128], f8, tag=f"w1e0_{q}",
                                       name=f"w1e0_{q}")
                        nc.sync.dma_start(
                            t1[:],
                            din["we1"][0, q].rearrange("p (i a n) -> p i a n",
                                                       i=4, a=DC))
                        w1e0.append(t1)
                        t2 = wpre.tile([128, 4, D], f8, tag=f"w2e0_{q}",
                                       name=f"w2e0_{q}")
                        nc.sync.dma_start(
                            t2[:],
                            din["we2"][0, q].rearrange("p (i n) -> p i n",
                                                       i=4))
                        w2e0.append(t2)
                    ctmp = sC.enter_context(tc.tile_pool(name="ctmp", bufs=2))
                    cps = sC.enter_context(
                        tc.tile_pool(name="pc_ps", bufs=2, space="PSUM"))
                    for tt in range(8):
                        xo = ctmp.tile([128, D], f32, tag="xo", name="xo")
                        nc.sync.dma_start(
                            xo[:], din["xp"][tt * 128:(tt + 1) * 128, :])
                        x1 = cpersist.tile([128, D], f32, tag=f"x1_{tt}",
                                           name=f"x1_{tt}")
                        for nb in range(2):
                            yp = cps.tile([128, 384], f32, tag="yp", name="yp")
                            for dc in range(DC):
                                nc.tensor.matmul(
                                    yp[:],
                                    attT[:, dc, tt * 128:(tt + 1) * 128],
                                    wo_sb[:, dc, nb * 384:(nb + 1) * 384],
                                    start=(dc == 0), stop=(dc == DC - 1))
                            nc.vector.tensor_add(
                                x1[:, nb * 384:(nb + 1) * 384], yp[:],
                                xo[:, nb * 384:(nb + 1) * 384])
                        nc.sync.dma_start(x1dbg[tt * 128:(tt + 1) * 128, :],
                                          x1[:])
                        xn2raw = _ln(nc, ctmp, x1, eps_col, xo)
                        xn2 = cpersist.tile([128, D], f32, tag=f"xn2_{tt}",
                                            name=f"xn2_{tt}")
                        nc.vector.tensor_copy(xn2[:], xn2raw[:])
                        tp4 = cps.tile([128, 512], f32, tag="tp2", name="tp2")
                        for i in range(4):
                            nc.tensor.transpose(
                                tp4[:, i * 128:(i + 1) * 128],
                                xn2[:, i * 128:(i + 1) * 128], ident[:])
                        nc.vector.tensor_copy(
                            xn2T[:, 0:4, tt * 128:(tt + 1) * 128],
                            tp4[:].rearrange("p (a n) -> p a n", a=4))
                        tp2 = cps.tile([128, 256], f32, tag="tp2c",
                                       name="tp2c")
                        for i in range(2):
                            nc.tensor.transpose(
                                tp2[:, i * 128:(i + 1) * 128],
                                xn2[:, (4 + i) * 128:(5 + i) * 128], ident[:])
                        nc.vector.tensor_copy(
                            xn2T[:, 4:6, tt * 128:(tt + 1) * 128],
                            tp2[:].rearrange("p (a n) -> p a n", a=2))
                        x1_t.append(x1)
                        xn2_t.append(xn2)

            # ---------- Phase D: router ----------
            rpool = top.enter_context(tc.tile_pool(name="rpool", bufs=1,
                                                   side="right"))
            rps = sCF.enter_context(tc.tile_pool(name="pd_ps", bufs=1,
                                                 space="PSUM"))
            cnt_ps = rps.tile([9, 8], f32)
            with ExitStack() as pd:
                dps = pd.enter_context(
                    tc.tile_pool(name="pd_ps2", bufs=2, space="PSUM"))
                dpool = pd.enter_context(tc.tile_pool(name="pd_tmp", bufs=2))
                dkeep = pd.enter_context(tc.tile_pool(name="pd_keep", bufs=1))
                rt_t, zp1_t, sp0_t, nt_t = [], [], [], []
                # pass 1: noise DMAs + router logits + Exp (one table)
                for tt in range(8):
                    nt = dkeep.tile([128, 8], f32, tag=f"nt_{tt}",
                                    name=f"nt_{tt}")
                    nc.sync.dma_start(
                        nt[:], din["noise"][tt * 128:(tt + 1) * 128, :])
                    nt_t.append(nt)
                for tt in range(8):
                    rp = dps.tile([128, 17], f32, tag="rp", name="rp")
                    for dc in range(DC):
                        nc.tensor.matmul(
                            rp[:], xn2T[:, dc, tt * 128:(tt + 1) * 128],
                            wrout_sb[:, dc, :],
                            start=(dc == 0), stop=(dc == DC - 1))
                    rt = dkeep.tile([128, 17], f32, tag=f"rt_{tt}",
                                    name=f"rt_{tt}")
                    nc.vector.tensor_add(rt[:], rp[:], rbias_bc[:])
                    z = dpool.tile([128, 8], f32, tag="z", name="z")
                    nc.scalar.activation(z[:], rt[:, 8:16], AF.Exp)
                    zp1 = dkeep.tile([128, 8], f32, tag=f"zp1_{tt}",
                                     name=f"zp1_{tt}")
                    nc.vector.tensor_scalar_add(zp1[:], z[:], 1.0)
                    rt_t.append(rt); zp1_t.append(zp1)
                # pass 2: Ln (one table)
                for tt in range(8):
                    sp0 = dkeep.tile([128, 8], f32, tag=f"sp0_{tt}",
                                     name=f"sp0_{tt}")
                    nc.scalar.activation(sp0[:], zp1_t[tt][:], AF.Ln)
                    sp0_t.append(sp0)
                # pass 3: Exp (one table) + vector chain
                for tt in range(8):
                    rt, zp1, sp0, nt = (rt_t[tt], zp1_t[tt], sp0_t[tt],
                                        nt_t[tt])
                    en = dpool.tile([128, 8], f32, tag="en", name="en")
                    nc.scalar.activation(en[:], sp0[:], AF.Exp, scale=-1.0)
                    t1 = dpool.tile([128, 8], f32, tag="t1", name="t1")
                    nc.vector.tensor_tensor(out=t1[:], in0=zp1[:], in1=en[:],
                                            op=ALU.mult)
                    nc.vector.tensor_scalar_add(t1[:], t1[:], -1.0)
                    sp = dpool.tile([128, 8], f32, tag="sp", name="sp")
                    nc.vector.tensor_add(sp[:], sp0[:], t1[:])
                    nm = dpool.tile([128, 8], f32, tag="nm", name="nm")
                    nc.vector.tensor_tensor(out=nm[:], in0=nt[:], in1=sp[:],
                                            op=ALU.mult)
                    noisy = dpool.tile([128, 8], f32, tag="noisy", name="noisy")
                    nc.vector.tensor_add(noisy[:], rt[:, 0:8], nm[:])
                    t8 = dpool.tile([128, 8], f32, tag="t8", name="t8")
                    nc.vector.max(t8[:], noisy[:])
                    ix = dpool.tile([128, 8], u32, tag="ix", name="ix")
                    nc.vector.max_index(ix[:], t8[:], noisy[:])
                    ixf = dpool.tile([128, 8], f32, tag="ixf", name="ixf")
                    nc.vector.tensor_copy(ixf[:], ix[:])
                    dv = dpool.tile([128, 1], f32, tag="dv", name="dv")
                    nc.vector.tensor_sub(dv[:], t8[:, 1:2], t8[:, 0:1])
                    ge = dpool.tile([128, 1], f32, tag="ge", name="ge")
                    nc.scalar.activation(ge[:], dv[:], AF.Exp)
                    gp1 = dpool.tile([128, 1], f32, tag="gp1", name="gp1")
                    nc.vector.tensor_scalar_add(gp1[:], ge[:], 1.0)
                    g1 = rpool.tile([128, 1], f32, tag=f"g1_{tt}",
                                    name=f"g1_{tt}")
                    nc.vector.reciprocal(g1[:], gp1[:])
                    g2 = rpool.tile([128, 1], f32, tag=f"g2_{tt}",
                                    name=f"g2_{tt}")
                    nc.vector.tensor_tensor(out=g2[:], in0=ge[:], in1=g1[:],
                                            op=ALU.mult)
                    ns = rpool.tile([128, 1], f32, tag=f"ns_{tt}",
                                    name=f"ns_{tt}")
                    nc.vector.tensor_scalar(ns[:], rt[:, 16:17], 0.0, None,
                                            ALU.is_le)
                    m0 = rpool.tile([128, 8], f32, tag=f"m0_{tt}",
                                    name=f"m0_{tt}")
                    nc.vector.tensor_scalar(m0[:], iota8[:], ixf[:, 0:1], None,
                                            ALU.is_equal)
                    m1 = rpool.tile([128, 8], f32, tag=f"m1_{tt}",
                                    name=f"m1_{tt}")
                    nc.vector.tensor_scalar(m1[:], iota8[:], ixf[:, 1:2], None,
                                            ALU.is_equal)
                    gf = rpool.tile([128, 8], f32, tag=f"gf_{tt}",
                                    name=f"gf_{tt}")
                    ga = dpool.tile([128, 8], f32, tag="ga", name="ga")
                    nc.vector.tensor_scalar(ga[:], m0[:], g1[:, :1], None,
                                            ALU.mult)
                    gb = dpool.tile([128, 8], f32, tag="gb", name="gb")
                    nc.vector.tensor_scalar(gb[:], m1[:], g2[:, :1], None,
                                            ALU.mult)
                    nc.vector.tensor_add(gf[:], ga[:], gb[:])
                    m = rpool.tile([128, 8], f32, tag=f"m_{tt}", name=f"m_{tt}")
                    nc.vector.tensor_add(m[:], m0[:], m1[:])
                    nc.vector.tensor_scalar_min(m[:], m[:], 1.0)
                    nc.vector.tensor_scalar(m[:], m[:], ns[:, :1], None,
                                            ALU.mult)
                    maug = rpool.tile([128, 9], f32, tag=f"maug_{tt}",
                                      name=f"maug_{tt}")
                    nc.vector.tensor_copy(maug[:, 0:8], m[:])
                    nc.vector.tensor_copy(maug[:, 8:9], ns[:])
                    nc.tensor.matmul(cnt_ps[:, tt:tt + 1], maug[:], ones_r[:],
                                     start=True, stop=True)
                    g1p = rpool.tile([128, 1], f32, tag=f"g1p_{tt}",
                                     name=f"g1p_{tt}")
                    nc.vector.tensor_tensor(out=g1p[:], in0=g1[:], in1=ns[:],
                                            op=ALU.mult)
                    g2p = rpool.tile([128, 1], f32, tag=f"g2p_{tt}",
                                     name=f"g2p_{tt}")
                    nc.vector.tensor_tensor(out=g2p[:], in0=g2[:], in1=ns[:],
                                            op=ALU.mult)
                    nsp = rpool.tile([128, 1], f32, tag=f"nsp_{tt}",
                                     name=f"nsp_{tt}")
                    nc.vector.tensor_scalar(nsp[:], ns[:], 1.0, -1.0,
                                            ALU.subtract, ALU.mult)
                    g1_t.append(g1p); g2_t.append(g2p); ns_t.append(nsp)
                    m0_t.append(m0); m1_t.append(m1); m_t.append(m)
                    maug_t.append(maug); gf_t.append(gf)

            cnt_sb = rpool.tile([9, 8], f32)
            nc.vector.tensor_copy(cnt_sb[:], cnt_ps[:])
            nc.sync.dma_start(cc_in[:], cnt_sb[:])
            nc.gpsimd.collective_compute(
                "AllGather", ALU.bypass, replica_groups=[list(range(NCORES))],
                ins=[cc_in.opt()], outs=[cc_out.opt()])
            # ---------- F-LOCAL: candidate-rank slots + scatter ----------
            # (no dependency on the AllGather: slots are local candidate
            # ranks; the capacity test moves after the collective, under G)
            zf8 = rpool.tile([8, 8], f32)
            nc.vector.memset(zf8[:], 0.0)
            cincl = rpool.tile([8, 8], f32)
            nc.vector.tensor_tensor_scan(cincl[:], cnt_sb[0:8, :], zf8[:],
                                         0.0, ALU.add, ALU.add)
            cexcl = rpool.tile([8, 8], f32)
            nc.vector.tensor_sub(cexcl[:], cincl[:], cnt_sb[0:8, :])
            cexT_ps = rps.tile([8, 8], f32)
            nc.tensor.transpose(cexT_ps[:], cexcl[:], ident[0:8, 0:8])
            cexT = rpool.tile([8, 8], f32)
            nc.vector.tensor_copy(cexT[:], cexT_ps[:])

            crow_t, slotf_t, fi_t = [], [], []
            with ExitStack() as pf2:
                f2ps = pf2.enter_context(
                    tc.tile_pool(name="pf2_ps", bufs=2, space="PSUM"))
                f2p = pf2.enter_context(tc.tile_pool(name="pf2_sb", bufs=2))
                for tt in range(8):
                    p2 = f2ps.tile([128, 8], f32, tag="p2", name="p2")
                    crow = rpool.tile([1, 8], f32, tag=f"crow_{tt}",
                                      name=f"crow_{tt}")
                    nc.sync.dma_start(crow[:], cexT[tt:tt + 1, :])
                    nc.tensor.matmul(p2[:], ones1r[:], crow[:],
                                     start=True, stop=False)
                    nc.tensor.matmul(p2[:], lstrict[:], m_t[tt][:],
                                     start=False, stop=True)
                    slotf = rpool.tile([128, 8], f32, tag=f"slotf_{tt}",
                                       name=f"slotf_{tt}")
                    nc.vector.tensor_add(slotf[:], p2[:], iotase[:])
                    fi_k = []
                    for k, mk in ((0, m0_t[tt]), (1, m1_t[tt])):
                        fim = f2p.tile([128, 8], f32, tag="fim", name="fim")
                        nc.vector.tensor_tensor(out=fim[:], in0=slotf[:],
                                                in1=mk[:], op=ALU.mult)
                        fi = rpool.tile([128, 1], f32, tag=f"fi_{tt}_{k}",
                                        name=f"fi_{tt}_{k}")
                        nc.vector.reduce_sum(fi[:], fim[:],
                                             axis=mybir.AxisListType.X)
                        cm = f2p.tile([128, 8], f32, tag="cm", name="cm")
                        nc.vector.tensor_tensor(out=cm[:], in0=mk[:],
                                                in1=m_t[tt][:], op=ALU.mult)
                        candk = f2p.tile([128, 1], f32, tag="candk",
                                         name="candk")
                        nc.vector.reduce_sum(candk[:], cm[:],
                                             axis=mybir.AxisListType.X)
                        u = f2p.tile([128, 1], f32, tag="u", name="u")
                        nc.vector.tensor_scalar_add(u[:], candk[:], -1.0)
                        nc.vector.tensor_scalar_mul(u[:], u[:], -70000.0)
                        fis = f2p.tile([128, 1], f32, tag="fis", name="fis")
                        nc.vector.tensor_add(fis[:], fi[:], u[:])
                        fii = f2p.tile([128, 1], i32, tag="fii", name="fii")
                        nc.vector.tensor_copy(fii[:], fis[:])
                        nc.gpsimd.indirect_dma_start(
                            out=xe_dram[:],
                            out_offset=bass.IndirectOffsetOnAxis(
                                ap=fii[:, :1], axis=0),
                            in_=xn2_t[tt][:], in_offset=None,
                            bounds_check=E * SE - 1, oob_is_err=False)
                        fi_k.append(fi)
                    crow_t.append(crow); slotf_t.append(slotf)
                    fi_t.append(fi_k)

        # ---------- Phase G: expert MLPs ----------
        zrow = rpool.tile([128, D], f32)
        nc.vector.memset(zrow[:], 0.0)
        nc.sync.dma_start(ye_dram[ZROW:ZROW + 1, :], zrow[0:1, :])
        with ExitStack() as pg:
            gsb = pg.enter_context(tc.tile_pool(name="pg_sb", bufs=2))
            xetp = pg.enter_context(tc.tile_pool(name="pg_xet", bufs=1))
            w1p = pg.enter_context(tc.tile_pool(name="pg_w1", bufs=12))
            w2p = pg.enter_context(tc.tile_pool(name="pg_w2", bufs=12))
            hpool = pg.enter_context(tc.tile_pool(name="pg_h", bufs=2))
            ROWS = [(0, 128), (128, 64)]
            xet = xetp.tile([128, E, DC, SE], f8)
            with ExitStack() as pgt:
                tps = pgt.enter_context(
                    tc.tile_pool(name="pg_tps", bufs=2, space="PSUM"))
                for e in range(E):
                    for r0, rn in ROWS:
                        xe = gsb.tile([128, D], f32, tag="xe", name="xe")
                        nc.sync.dma_start(
                            xe[0:rn, :],
                            xe_dram[e * SE + r0:e * SE + r0 + rn, :])
                        tp4 = tps.tile([128, 512], f32, tag="tp3", name="tp3")
                        for i in range(4):
                            nc.tensor.transpose(
                                tp4[:, i * 128:i * 128 + rn],
                                xe[0:rn, i * 128:(i + 1) * 128],
                                ident[0:rn, 0:rn])
                        nc.vector.tensor_scalar_mul(
                            xet[:, e, 0:4, r0:r0 + rn],
                            tp4[:].rearrange("p (a n) -> p a n",
                                             a=4)[:, :, 0:rn],
                            XSCALE)
                        tp2 = tps.tile([128, 256], f32, tag="tp3b",
                                       name="tp3b")
                        for i in range(2):
                            nc.tensor.transpose(
                                tp2[:, i * 128:i * 128 + rn],
                                xe[0:rn, (4 + i) * 128:(5 + i) * 128],
                                ident[0:rn, 0:rn])
                        nc.vector.tensor_scalar_mul(
                            xet[:, e, 4:6, r0:r0 + rn],
                            tp2[:].rearrange("p (a n) -> p a n",
                                             a=2)[:, :, 0:rn],
                            XSCALE)
            gps = pg.enter_context(
                tc.tile_pool(name="pg_ps", bufs=2, space="PSUM"))
            yps = pg.enter_context(
                tc.tile_pool(name="pg_yps", bufs=1, space="PSUM"))
            def emit_late():
                # post-collective capacity/keep path (runs under Phase G)
                cnts_all = rpool.tile([72, 8], f32)
                nc.sync.dma_start(cnts_all[:], cc_out[:])
                flat = rpool.tile([9, 64], f32)
                for r in range(NCORES):
                    b2, a = r // 2, r % 2
                    if a == 0:
                        nc.sync.dma_start(flat[:, b2 * 16:b2 * 16 + 4],
                                          cnts_all[9 * r:9 * r + 9, 0:4])
                        nc.sync.dma_start(flat[:, b2 * 16 + 12:b2 * 16 + 16],
                                          cnts_all[9 * r:9 * r + 9, 4:8])
                    else:
                        nc.sync.dma_start(flat[:, b2 * 16 + 4:b2 * 16 + 12],
                                          cnts_all[9 * r:9 * r + 9, 0:8])
                zf = rpool.tile([9, 64], f32)
                nc.vector.memset(zf[:], 0.0)
                incl = rpool.tile([9, 64], f32)
                nc.vector.tensor_tensor_scan(incl[:], flat[:], zf[:], 0.0,
                                             ALU.add, ALU.add)
                excl = rpool.tile([9, 64], f32)
                nc.vector.tensor_sub(excl[:], incl[:], flat[:])
                tot = rpool.tile([1, 1], f32)
                nc.sync.dma_start(tot[:], incl[8:9, 63:64])
                tot_i = rpool.tile([1, 1], i32)
                nc.vector.tensor_copy(tot_i[:], tot[:])
                cap_i = rpool.tile([1, 1], i32)
                nc.vector.tensor_scalar(cap_i[:], tot_i[:], 2, None,
                                        ALU.arith_shift_right)
                capt = rpool.tile([1, 1], f32)
                nc.vector.tensor_copy(capt[:], cap_i[:])
                cap_bc = rpool.tile([128, 1], f32)
                nc.gpsimd.partition_broadcast(cap_bc[:], capt[:])

                exT_ps = gps.tile([64, 9], f32, tag="lps", name="exT")
                nc.tensor.transpose(exT_ps[:], excl[:, 0:64],
                                    ident[0:9, 0:9])
                exT = rpool.tile([64, 9], f32)
                nc.vector.tensor_copy(exT[:], exT_ps[:])
                myo_ps = gps.tile([9, 8], f32, tag="lps", name="myo")
                nc.tensor.matmul(myo_ps[:], exT[:, 0:9], chsel_sb[:],
                                 start=True, stop=True)
                myo = rpool.tile([9, 8], f32)
                nc.vector.tensor_copy(myo[:], myo_ps[:])
                myoT_ps = gps.tile([8, 9], f32, tag="lps", name="myoT")
                nc.tensor.transpose(myoT_ps[:], myo[:], ident[0:9, 0:9])
                myoT = rpool.tile([8, 9], f32)
                nc.vector.tensor_copy(myoT[:], myoT_ps[:])

                for tt in range(8):
                    orow = rpool.tile([1, 8], f32, tag=f"orow_{tt}",
                                      name=f"orow_{tt}")
                    nc.sync.dma_start(orow[:], myoT[tt:tt + 1, 0:8])
                    d1 = rpool.tile([1, 8], f32, tag=f"d1_{tt}",
                                    name=f"d1_{tt}")
                    nc.vector.tensor_sub(d1[:], orow[:], crow_t[tt][:])
                    nc.vector.tensor_sub(d1[:], d1[:], iotase[0:1, :])
                    dbc = rpool.tile([128, 8], f32, tag=f"dbc_{tt}",
                                     name=f"dbc_{tt}")
                    nc.gpsimd.partition_broadcast(dbc[:], d1[:])
                    # pr = slotf + m + (orow - crow - iotase)
                    pr = rpool.tile([128, 8], f32, tag=f"pr_{tt}",
                                    name=f"pr_{tt}")
                    nc.vector.tensor_add(pr[:], slotf_t[tt][:], m_t[tt][:])
                    nc.vector.tensor_add(pr[:], pr[:], dbc[:])
                    keepb = rpool.tile([128, 8], f32, tag=f"kb_{tt}",
                                       name=f"kb_{tt}")
                    nc.vector.tensor_scalar(keepb[:], pr[:], cap_bc[:, :1],
                                            None, ALU.is_le)
                    keep = rpool.tile([128, 8], f32, tag=f"keep_{tt}",
                                      name=f"keep_{tt}")
                    nc.vector.tensor_tensor(out=keep[:], in0=keepb[:],
                                            in1=m_t[tt][:], op=ALU.mult)
                    kg = rpool.tile([128, 8], f32, tag=f"kg_{tt}",
                                    name=f"kg_{tt}")
                    nc.vector.tensor_tensor(out=kg[:], in0=keep[:],
                                            in1=gf_t[tt][:], op=ALU.mult)
                    nc.sync.dma_start(rdbg[tt * 128:(tt + 1) * 128, :],
                                      kg[:])
                    gidx = rpool.tile([128, 2], i32, tag=f"gi_{tt}",
                                      name=f"gi_{tt}")
                    for k, mk in ((0, m0_t[tt]), (1, m1_t[tt])):
                        km = rpool.tile([128, 8], f32, tag="late_km",
                                        name="late_km")
                        nc.vector.tensor_tensor(out=km[:], in0=mk[:],
                                                in1=keep[:], op=ALU.mult)
                        kept = rpool.tile([128, 1], f32, tag="late_kept",
                                          name="late_kept")
                        nc.vector.reduce_sum(kept[:], km[:],
                                             axis=mybir.AxisListType.X)
                        gi = rpool.tile([128, 1], f32, tag="late_gi",
                                        name="late_gi")
                        nc.vector.tensor_tensor(out=gi[:], in0=fi_t[tt][k][:],
                                                in1=kept[:], op=ALU.mult)
                        w = rpool.tile([128, 1], f32, tag="late_w",
                                       name="late_w")
                        nc.vector.tensor_scalar_add(w[:], kept[:], -1.0)
                        nc.vector.tensor_scalar_mul(w[:], w[:], -float(ZROW))
                        nc.vector.tensor_add(gi[:], gi[:], w[:])
                        nc.vector.tensor_copy(gidx[:, k:k + 1], gi[:])
                    gidx_t.append(gidx)

            for e in range(E):
                if e == 2:
                    emit_late()
                hT = hpool.tile([128, 24, SE], f8, tag="hT", name="hT")
                for q in range(6):
                    if e == 0:
                        w1t = w1e0[q]
                    else:
                        w1t = w1p.tile([128, 4, DC, 128], f8, tag="w1t",
                                       name="w1t")
                        nc.sync.dma_start(
                            w1t[:],
                            din["we1"][e, q].rearrange(
                                "p (i a n) -> p i a n", i=4, a=DC))
                    for i in range(4):
                        mt = 4 * q + i
                        hp = gps.tile([128, SE], f32, tag="hp", name="hp")
                        for dh in range(3):
                            nc.tensor.matmul(
                                hp[:], w1t[:, i, 2 * dh:2 * dh + 2, :],
                                xet[:, e, 2 * dh:2 * dh + 2, :],
                                start=(dh == 0), stop=(dh == 2),
                                perf_mode=DR)
                        nc.vector.tensor_scalar(hT[:, mt, :], hp[:], 0.0,
                                                1.0 / WSCALE, ALU.max,
                                                ALU.mult)
                ypl = [yps.tile([128, 384], f32, tag=f"yp_{i}",
                                name=f"ypl_{i}") for i in range(4)]
                for q in range(6):
                    if e == 0:
                        w2t = w2e0[q]
                    else:
                        w2t = w2p.tile([128, 4, D], f8, tag="w2t",
                                       name="w2t")
                        nc.sync.dma_start(
                            w2t[:],
                            din["we2"][e, q].rearrange("p (i n) -> p i n",
                                                       i=4))
                    for ip in range(2):
                        hc0 = 4 * q + 2 * ip
                        for rt2, (r0, rn) in enumerate(ROWS):
                            for nb in range(2):
                                nc.tensor.matmul(
                                    ypl[rt2 * 2 + nb][0:rn, :],
                                    hT[:, hc0:hc0 + 2, r0:r0 + rn],
                                    w2t[:, 2 * ip:2 * ip + 2,
                                        nb * 384:(nb + 1) * 384],
                                    start=(hc0 == 0), stop=(hc0 == 22),
                                    perf_mode=DR)
                for rt2, (r0, rn) in enumerate(ROWS):
                    ysb = gsb.tile([128, D], f32, tag="ysb", name="ysb")
                    for nb in range(2):
                        nc.vector.tensor_scalar_mul(
                            ysb[0:rn, nb * 384:(nb + 1) * 384],
                            ypl[rt2 * 2 + nb][0:rn, :],
                            1.0 / (XSCALE * WSCALE))
                    nc.sync.dma_start(
                        ye_dram[e * SE + r0:e * SE + r0 + rn, :],
                        ysb[0:rn, :])

        # ---------- Phase H ----------
        with ExitStack() as ph:
            hsb = ph.enter_context(tc.tile_pool(name="ph_sb", bufs=3))
            for tt in range(8):
                yg0 = hsb.tile([128, D], f32, tag="yg0", name="yg0")
                nc.gpsimd.indirect_dma_start(
                    out=yg0[:], out_offset=None, in_=ye_dram[:],
                    in_offset=bass.IndirectOffsetOnAxis(
                        ap=gidx_t[tt][:, 0:1], axis=0))
                yg1 = hsb.tile([128, D], f32, tag="yg1", name="yg1")
                nc.gpsimd.indirect_dma_start(
                    out=yg1[:], out_offset=None, in_=ye_dram[:],
                    in_offset=bass.IndirectOffsetOnAxis(
                        ap=gidx_t[tt][:, 1:2], axis=0))
                u0 = hsb.tile([128, D], f32, tag="u0", name="u0")
                nc.vector.tensor_scalar(u0[:], yg0[:], g1_t[tt][:, :1], None,
                                        ALU.mult)
                u1 = hsb.tile([128, D], f32, tag="u1", name="u1")
                nc.scalar.activation(u1[:], yg1[:], AF.Copy,
                                     scale=g2_t[tt][:, :1])
                w = hsb.tile([128, D], f32, tag="w", name="w")
                nc.vector.tensor_scalar(w[:], xn2_t[tt][:], ns_t[tt][:, :1],
                                        None, ALU.mult)
                s01 = hsb.tile([128, D], f32, tag="s01", name="s01")
                nc.vector.tensor_add(s01[:], u0[:], u1[:])
                wx = hsb.tile([128, D], f32, tag="wx", name="wx")
                nc.vector.tensor_add(wx[:], w[:], x1_t[tt][:])
                out = hsb.tile([128, D], f32, tag="out", name="out")
                nc.vector.tensor_add(out[:], s01[:], wx[:])
                nc.sync.dma_start(yout[tt * 128:(tt + 1) * 128, :], out[:])

    nc.compile()
    return nc


_OWN = {0: [0, 1, 2, 3, 12, 13, 14, 15], 1: [4, 5, 6, 7, 8, 9, 10, 11]}


def _core_meta(c):
    b, a = c // 2, c % 2
    own = _OWN[a]
    other = [g for g in range(16) if g not in own]
    perm_chunks = own + other
    rows = np.concatenate([np.arange(g * 128, (g + 1) * 128)
                           for g in perm_chunks])
    return b, a, own, rows


def _host_inputs(x, noise, Wq, Wk, Wv, Wo, Wr, br, Wn, bn, Wsk, bsk, We1, We2,
                 **_unused):
    x = np.asarray(x, np.float32)
    noise = np.asarray(noise, np.float32)
    wq = np.ascontiguousarray(
        (np.transpose(np.asarray(Wq), (1, 0, 2)).reshape(D, D)
         * np.float32(D ** -0.5)).astype(np.float32))
    wk = np.ascontiguousarray(
        np.transpose(np.asarray(Wk), (1, 0, 2)).reshape(D, D)
        .astype(np.float32))
    wqh = np.ascontiguousarray(
        wq.reshape(DC, 128, H, HS).transpose(2, 1, 0, 3)
        .reshape(H, 128, DC * HS))
    wkh = np.ascontiguousarray(
        wk.reshape(DC, 128, H, HS).transpose(2, 1, 0, 3)
        .reshape(H, 128, DC * HS))
    wv = np.ascontiguousarray(
        np.transpose(np.asarray(Wv), (1, 0, 2)).reshape(D, D)
        .astype(np.float32))
    wrout = np.ascontiguousarray(np.concatenate(
        [np.asarray(Wr), np.asarray(Wn), np.asarray(Wsk)], axis=1)
        .astype(np.float32))
    rbias = np.concatenate(
        [np.asarray(br), np.asarray(bn), np.asarray(bsk)])[None, :] \
        .astype(np.float32)
    qsel = np.zeros((2, NOWN), np.float32)
    qsel[0, 0:512] = 1.0
    qsel[1, 512:1024] = 1.0
    import ml_dtypes
    W1 = (np.asarray(We1, np.float32) * WSCALE).reshape(E, DC, 128, 6, 4, 128)
    we1 = np.ascontiguousarray(
        W1.transpose(0, 3, 2, 4, 1, 5).reshape(E, 6, 128, 3072)
        .astype(ml_dtypes.float8_e4m3))
    W2 = (np.asarray(We2, np.float32) * WSCALE).reshape(E, 6, 4, 128, D)
    we2 = np.ascontiguousarray(
        W2.transpose(0, 1, 3, 2, 4).reshape(E, 6, 128, 3072)
        .astype(ml_dtypes.float8_e4m3))
    wo = np.ascontiguousarray(np.asarray(Wo, np.float32))

    in_maps = []
    for c in range(NCORES):
        b, a, own, rows = _core_meta(c)
        gid = rows
        kbias = np.zeros((2, T), np.float32)
        for s in range(2):
            qmax = gid[s * 512:(s + 1) * 512].max()
            kbias[s] = np.where(gid > qmax, NEG, 0.0).astype(np.float32)
        chunksel = np.zeros((64, E), np.float32)
        for lc in range(8):
            chunksel[b * 16 + own[lc], lc] = 1.0
        in_maps.append({
            "xp": np.ascontiguousarray(x[b][rows]),
            "noise": np.ascontiguousarray(noise[b][rows[:NOWN]]),
            "wqh": wqh, "wkh": wkh, "wv": wv, "wo": wo,
            "wrout": wrout, "rbias": rbias,
            "we1": we1, "we2": we2,
            "kbias": kbias, "qsel": qsel,
            "chunksel": chunksel,
        })
    return in_maps


_prog = None


def run(trace=False, **inputs):
    global _prog
    if _prog is None:
        _prog = build_program()
    in_maps = _host_inputs(**inputs)
    res = bass_utils.run_bass_kernel_spmd(
        _prog, in_maps, core_ids=list(range(NCORES)), trace=trace)
    out = np.zeros((B, T, D), np.float32)
    for c in range(NCORES):
        b, a, own, rows = _core_meta(c)
        out[b][rows[:NOWN]] = res.results[c]["yout"]
    return out, res


def kernel(**inputs):
    out, _ = run(trace=False, **inputs)
    return out
